# revision 38
# baseline (speedup 1.0000x reference)
"""DeepSeek-style MoE forward on 8 Trainium2 NeuronCores.

Strategy (expert-parallel, per sharding hint):
  Phase 1 (device, data-parallel): each core computes the softmax gate +
    top-2 renormalized weights for its 512-token slice, in f16 with a
    sigmoid-based renormalization (w1 = sigmoid(s1 - s2), exact).
  Host dispatch: tokens are routed to expert cores by the device-computed
    top-k weights (the "all-to-all", emulated with numpy gathers; layout
    transposed to feature-major for the device).
  Phase 2 (device, expert-parallel): core e holds expert e's weights and
    runs the SwiGLU FFN on its gathered tokens; the routing weight is
    folded into hh before the down projection. Rare capacity overflow
    falls back to exact host math.
  Host combine: scatter-add expert outputs + residual.

Mixed fp8 precision (the big lever beyond the 240us fp16 baseline):
  The tensor engine runs fp8e4 (e4m3) matmuls at 2x rate via DoubleRow
  perf mode (two 128-deep k-subtiles per instruction, 0.5 cyc/row).
  Full-fp8 blows the 2e-2 accuracy budget (sim: 2.7e-2), so precision is
  configurable per stage and spent where it buys the most cycles:
    BASS_GU8: 8 bits, m-th H-column-tile of the gate/up projections in
      fp8 (x8 @ 32*w8, psum descaled in the silu/mul epilogue).
    BASS_DN8: 4 bits, p-th k-pair of the down projection in fp8
      (hh stored as e4m3(8*hh*wsel), wd8 = e4m3(32*wd); fp8 and fp16
      k-pairs accumulate in separate PSUM banks, combined in the
      epilogue: o = ps16 + ps8/256).
  Error (sim, validated within 0.3% of device at all16): all16 3.0e-3,
  dn8x4 1.62e-2, dn8x4+gu1 1.80e-2, dn8x4+gu2 1.96e-2, full8 2.7e-2.

Perf notes inherited from the fp16 baseline (still load-bearing):
  - top-2 renorm via sigmoid(s1-s2) on logits, no exp/divide.
  - CAP 1064 (max expert load 1063); token-dim groups (512,512,40) for
    gate/up, tail groups in separate full PSUM banks (interleaved
    accumulation groups sharing a bank corrupt).
  - PE warm-up matmuls at kernel start flip the HAM clock gate to 8/8.
  - Every transfer split across both HWDGE queues by partition halves,
    issued in consumption order; weight-column pools 4-deep.
  - Down-proj emits full-row output DMAs per 128-token tile; o_row is
    split into left/right half-tiles so the first half DMAs out while
    the second half computes; narrow tail ct tile last.
  - Trace facts (fp16 baseline): tensor busy 188.7us = throttled
    roofline (throttle_avg_util_limit 91% -> eff ~2.19 GHz); ~27us idle
    split between launch preamble (~7us fixed), DMA cold-start gaps and
    end drain. exec_time_ns = last_useful - first_useful on core 0.

Self-contained: shapes hardcoded from the problem spec.
"""
import os
import sys

import numpy as np
import ml_dtypes

if "/opt/trn_rl_repo" not in sys.path:
    sys.path.insert(0, "/opt/trn_rl_repo")

import concourse.tile as tile
from concourse import bacc, mybir
from concourse.bass_utils import run_bass_kernel_spmd
from concourse.masks import make_identity

B, S, D, E, H = 2, 2048, 2048, 8, 1024
T = B * S            # 4096 tokens
N_CORES = 8
TPC = T // N_CORES   # 512 tokens/core for the gate phase
CAP = int(os.environ.get("BASS_MOE_CAP", "1064"))  # per-expert capacity
P = 128
KD = D // P          # 16
KH = H // P          # 8
GU_FULL = [(i * 512, 512) for i in range(CAP // 512)]
GU_TAIL = (CAP - CAP % 512, CAP % 512) if CAP % 512 else None
GU_TAIL_N = GU_TAIL[1] if GU_TAIL else 0
CT_TILES = []
_c0 = 0
while _c0 < CAP:
    CT_TILES.append((_c0, min(P, CAP - _c0)))
    _c0 += P
DG = 512             # down-proj free-dim group
F32 = mybir.dt.float32
F16 = mybir.dt.float16
F8 = mybir.dt.float8e4
WNP = np.float16
E4NP = ml_dtypes.float8_e4m3
AF = mybir.ActivationFunctionType
OP = mybir.AluOpType
AX = mybir.AxisListType
PM_DR = mybir.MatmulPerfMode.DoubleRow

# ---- mixed-precision config ----
GU8 = [c == "1" for c in os.environ.get("BASS_GU8", "00000001")]
DN8 = [c == "1" for c in os.environ.get("BASS_DN8", "1111")]
assert len(GU8) == KH and len(DN8) == KH // 2
SW = 32.0            # fp8 weight scale (w*SW quantized to e4m3)
SH = 8.0             # fp8 hh scale
# DoubleRow LDWEIGHTS requires the stationary k-pair dim step % 16 == 0:
# fp8 token-major tiles are padded from CAP to CAP8
CAP8 = ((CAP + 15) // 16) * 16
K8 = [k for p in range(4) if DN8[p] for k in (2 * p, 2 * p + 1)]
K16 = [k for p in range(4) if not DN8[p] for k in (2 * p, 2 * p + 1)]
KD8N, KD16N = len(K8), len(K16)
M8 = [m for m in range(KH) if GU8[m]]
M16 = [m for m in range(KH) if not GU8[m]]
N8M, N16M = len(M8), len(M16)
M8_IDX = {m: i for i, m in enumerate(M8)}
M16_IDX = {m: i for i, m in enumerate(M16)}
# wsel variant per m: u-epilogue multiplier (SH if down-fp8) / (SW if gu-fp8)
_MSCALE = [
    (SH if DN8[m // 2] else 1.0) / (SW if GU8[m] else 1.0) for m in range(KH)
]
VS = sorted(set(_MSCALE))
NV = len(VS)
VIDX = [VS.index(s) for s in _MSCALE]
# xt chunking in k-tiles; fp8 chunks must have even widths/starts so
# DoubleRow k-pairs never straddle a chunk boundary
XT16_CH = [1, 1, 2, 4, 4, 4]
XT8_CH = [2, 2, 4, 4, 4]

_gate_nc = None
_moe_nc = None
_wprep_cache = {}
LAST_EXEC_NS = {"gate": None, "moe": None}
_TMPDIR = os.environ.get("BASS_KERNEL_TMPDIR")


def _axon_reset():
    """Recover a wedged NeuronCore via the axon client's reset entry
    point. Best-effort."""
    try:
        import ctypes

        lib = ctypes.CDLL("/opt/axon/libaxon_pjrt.so")
        lib.axon_reset.restype = ctypes.c_int64
        lib.axon_reset()
    except Exception:
        pass


_run_counter = [0]


def _run_spmd(nc, in_maps, trace, tag):
    _run_counter[0] += 1
    tag = f"{tag}_{_run_counter[0]}"
    try:
        return run_bass_kernel_spmd(
            nc, in_maps, core_ids=list(range(N_CORES)), trace=trace,
            tmpdir=(_TMPDIR + "/" + tag) if (trace and _TMPDIR) else None,
        )
    except Exception:
        _axon_reset()
        return run_bass_kernel_spmd(
            nc, in_maps, core_ids=list(range(N_CORES)), trace=trace,
            tmpdir=(_TMPDIR + "/" + tag + "_retry") if (trace and _TMPDIR) else None,
        )


def _emit_warmup(nc, sbuf_pool, psum_pool, psum_tag, n_mm, width, bufs=None,
                 act_funcs=(), mm_width=None):
    """Dummy matmuls with no DMA deps: keep the PE busy from the moment its
    preamble barrier clears so the HAM clock gate flips to 8/8 before the
    first real matmul. Also preloads activation tables (act_funcs) so the
    ~1.3us ACT_TABLE_LOAD overlaps the input DMA window instead of
    stalling the first real activation."""
    warm_src = sbuf_pool.tile([P, width], F16, tag="warm_src")
    nc.gpsimd.memset(warm_src[:], 0.0)
    if act_funcs:
        # separate tile: the act-table preload must not add a dependency
        # onto the warm-up matmuls' source
        warm_act = sbuf_pool.tile([1, 2], F32, tag="warm_act")
        nc.gpsimd.memset(warm_act[:], 0.0)
        for fn in act_funcs:
            nc.scalar.activation(warm_act[:, :1], warm_act[:, 1:], fn)
    mw = mm_width or width
    ps = psum_pool.tile([P, width], F32, tag=psum_tag, name="warm_ps", bufs=bufs)
    for _ in range(n_mm):
        nc.tensor.matmul(
            ps[:, :mw], lhsT=warm_src[:, :P], rhs=warm_src[:, :mw],
            start=True, stop=True,
        )


def _build_gate_nc():
    """Gate kernel: per-core 512-token slice -> renormalized top-2 weights.

    Inputs (feature-major, host-transposed f16 layout):
      xst  [P, KD, TPC]  slice of x^T   (xst[p, k, t] = x[t, k*P+p])
      wgt  [P, KD, E]    W_gate^T      (wgt[p, k, e] = W_gate[e, k*P+p])
    Output:
      wout [TPC, E]  w[t, e] = renormalized top-2 weight, 0 if not selected

    scores^T = wgt.T @ x^T with the 8-column gate weight stationary, then
    PE-transposed back to token-major. Top-2 renormalization uses
    w1 = sigmoid(s1 - s2), w2 = sigmoid(s2 - s1) (the softmax denominator
    cancels), batched across all 4 token tiles in an 11-op chain.
    """
    nc = bacc.Bacc(None, target_bir_lowering=False, enable_partition_id=False)
    xst = nc.dram_tensor("xst", [P, KD, TPC], F16, kind="ExternalInput")
    wgt = nc.dram_tensor("wgt", [P, KD, E], F16, kind="ExternalInput")
    wout = nc.dram_tensor("wout", [TPC, E], F32, kind="ExternalOutput")
    NTT = TPC // P  # 4 token tiles

    with tile.TileContext(nc) as tc:
        with (
            tc.tile_pool(name="xp", bufs=1) as xp,
            tc.tile_pool(name="wp", bufs=1) as wp,
            tc.tile_pool(name="psum", bufs=2, space="PSUM") as psum_pool,
            tc.tile_pool(name="v", bufs=2) as vp,
        ):
            _emit_warmup(nc, wp, psum_pool, "warm", 5, 256,
                         act_funcs=(AF.Sigmoid,))
            ident = wp.tile([P, P], F32)
            make_identity(nc, ident[:])
            wgt_sb = wp.tile([P, KD, E], F16)
            nc.sync.dma_start(wgt_sb[:], wgt[:])
            HP = P // 2
            # xst in 8 chunks of 2 k-tiles, each split across both HWDGE
            # queues by partition halves (disjoint SDMA engine sets);
            # small chunks keep the matmul stream fed from the first one
            CH = [2] * 8
            xst_ch = []
            k0 = 0
            for ci, w in enumerate(CH):
                t = xp.tile([P, w, TPC], F16, tag=f"xst{ci}", name=f"xst{ci}")
                s = xst[:, k0:k0 + w, :]
                nc.sync.dma_start(t[0:HP], s[0:HP])
                nc.scalar.dma_start(t[HP:P], s[HP:P])
                xst_ch.append((t, k0, w))
                k0 += w

            # scores^T [E, TPC], contraction over D in 16 k-tiles
            ps_st = psum_pool.tile([E, TPC], F32, tag="ps_st")
            for k in range(KD):
                t, k0, w = next(c for c in xst_ch if c[1] <= k < c[1] + c[2])
                nc.tensor.matmul(
                    ps_st[:],
                    lhsT=wgt_sb[:, k, :],
                    rhs=t[:, k - k0, :],
                    start=(k == 0),
                    stop=(k == KD - 1),
                )
            st_sb = vp.tile([E, TPC], F32, tag="st")
            nc.vector.tensor_copy(st_sb[:], ps_st[:])

            # transpose back to token-major: sc [P, NTT, E]
            sc = vp.tile([P, NTT, E], F32, tag="sc")
            for tt in range(NTT):
                ps = psum_pool.tile([P, E], F32, tag="scores")
                nc.tensor.transpose(
                    ps[:], st_sb[:, tt * P:(tt + 1) * P], ident[:E, :E]
                )
                nc.vector.tensor_copy(sc[:, tt, :], ps[:])

            # batched top-2 chain over [P, NTT, E]
            m1 = vp.tile([P, NTT, 1], F32, tag="m1")
            nc.vector.tensor_reduce(m1[:], sc[:], op=OP.max, axis=AX.X)
            mask1 = vp.tile([P, NTT, E], F32, tag="mask1")
            nc.vector.tensor_tensor(
                mask1[:], sc[:], m1[:].broadcast_to([P, NTT, E]), op=OP.is_equal
            )
            # sc2 = sc - 1e30*mask1 (knock out the max) in one op
            sc2 = vp.tile([P, NTT, E], F32, tag="sc2")
            nc.vector.scalar_tensor_tensor(
                sc2[:], mask1[:], -1e30, sc[:], op0=OP.mult, op1=OP.add
            )
            m2 = vp.tile([P, NTT, 1], F32, tag="m2")
            nc.vector.tensor_reduce(m2[:], sc2[:], op=OP.max, axis=AX.X)
            mask2 = vp.tile([P, NTT, E], F32, tag="mask2")
            nc.vector.tensor_tensor(
                mask2[:], sc2[:], m2[:].broadcast_to([P, NTT, E]), op=OP.is_equal
            )
            d12 = vp.tile([P, NTT, 1], F32, tag="d12")
            nc.vector.tensor_tensor(d12[:], m1[:], m2[:], op=OP.subtract)
            w1 = vp.tile([P, NTT, 1], F32, tag="w1")
            nc.scalar.activation(w1[:], d12[:], AF.Sigmoid)
            w2 = vp.tile([P, NTT, 1], F32, tag="w2")
            nc.scalar.activation(w2[:], d12[:], AF.Sigmoid, scale=-1.0)
            o1 = vp.tile([P, NTT, E], F32, tag="o1")
            nc.vector.tensor_tensor(
                o1[:], mask1[:], w1[:].broadcast_to([P, NTT, E]), op=OP.mult
            )
            w_all = vp.tile([P, NTT, E], F32, tag="w_all")
            nc.vector.tensor_tensor(
                w_all[:], mask2[:], w2[:].broadcast_to([P, NTT, E]), op=OP.mult
            )
            nc.vector.tensor_tensor(w_all[:], w_all[:], o1[:], op=OP.add)
            nc.scalar.dma_start(
                wout.rearrange("(tt p) e -> p tt e", p=P), w_all[:]
            )
    nc.compile()
    return nc


def _build_moe_nc():
    """Expert FFN kernel: out[c, :] = (silu(x_c @ Wg) * (x_c @ Wu) * wsel[c]) @ Wd.

    Inputs (host-prepared feature/contraction-major layouts; fp8 weights
    pre-scaled by SW, descaled on device):
      xt16   [P, KD, CAP]      f16  gathered tokens (if any fp16 m-tile)
      xt8    [P, KD, CAP]      f8   e4m3(x) (if any fp8 m-tile)
      wg16   [N16M, P, KD, P]  f16  gate-proj columns for fp16 m-tiles
      wg8    [N8M, P, KD, P]   f8   e4m3(SW*w) for fp8 m-tiles
      wu16/wu8                 likewise for up-proj
      wd16   [P, KD16N, D]     f16  down-proj k-slices for fp16 pairs
      wd8    [P, KD8N, D]      f8   e4m3(SW*wd) for fp8 pairs
      wselv  [P, NV, CAP]      f16  routing weight * per-m epilogue scale
    Output:
      out    [CAP, D] f16
    """
    nc = bacc.Bacc(None, target_bir_lowering=False, enable_partition_id=False)
    xt16 = nc.dram_tensor("xt16", [P, KD, CAP], F16, kind="ExternalInput") \
        if N16M else None
    xt8 = nc.dram_tensor("xt8", [P, KD, CAP8], F8, kind="ExternalInput") \
        if N8M else None
    wg16 = nc.dram_tensor("wg16", [N16M, P, KD, P], F16, kind="ExternalInput") \
        if N16M else None
    wu16 = nc.dram_tensor("wu16", [N16M, P, KD, P], F16, kind="ExternalInput") \
        if N16M else None
    wg8 = nc.dram_tensor("wg8", [N8M, P, KD, P], F8, kind="ExternalInput") \
        if N8M else None
    wu8 = nc.dram_tensor("wu8", [N8M, P, KD, P], F8, kind="ExternalInput") \
        if N8M else None
    wd16 = nc.dram_tensor("wd16", [P, KD16N, D], F16, kind="ExternalInput") \
        if KD16N else None
    wd8 = nc.dram_tensor("wd8", [P, KD8N, D], F8, kind="ExternalInput") \
        if KD8N else None
    wselv = nc.dram_tensor("wselv", [P, NV, CAP], F16, kind="ExternalInput")
    out = nc.dram_tensor("out", [CAP, D], F16, kind="ExternalOutput")

    with tile.TileContext(nc) as tc:
        with (
            tc.tile_pool(name="sb", bufs=1) as sb,
            tc.tile_pool(name="ps", bufs=1, space="PSUM") as ps_pool,
        ):
            misc = sb
            psum_pool = pgu_pool = ps_pool
            # warm-up psum shares the ps_tg tag: all 8 PSUM banks go to
            # ps_g/ps_u/ps_tg/ps_tu at bufs=2 (tail groups double-buffered
            # kills the ~0.8us stall at each m-tile boundary)
            _emit_warmup(nc, misc, ps_pool, "ps_tg", 10, 512, bufs=2,
                         act_funcs=(AF.Silu,), mm_width=256)

            HP = P // 2

            def dma2(dst, src):
                nc.sync.dma_start(dst[0:HP], src[0:HP])
                nc.scalar.dma_start(dst[HP:P], src[HP:P])

            def w_cols(m, first=False):
                if GU8[m]:
                    i = M8_IDX[m]
                    gsrc, usrc, dt8, tag = wg8[i], wu8[i], F8, "8"
                else:
                    i = M16_IDX[m]
                    gsrc, usrc, dt8, tag = wg16[i], wu16[i], F16, "16"
                sfx = "_c0" if first else ""
                g = sb.tile([P, KD, P], dt8, tag=f"wgcol{tag}",
                            name=f"wgcol{tag}{sfx}" if first else None, bufs=4)
                dma2(g[:], gsrc)
                u = sb.tile([P, KD, P], dt8, tag=f"wucol{tag}",
                            name=f"wucol{tag}{sfx}" if first else None, bufs=4)
                dma2(u[:], usrc)
                return g, u

            wg_c0, wu_c0 = w_cols(0, first=True)

            # xt chunk tiles; DMAs are placed individually: early chunks on
            # the two HWDGE queues in consumption order, the big tail
            # chunks and the late-consumed fp8 tokens on the gpsimd SWDGE
            # queue (3rd issue stream; all three share the 16 SDMA engines
            # but issue windows no longer serialize)
            def xt_alloc(dt, tag, width, chunks):
                ch = []
                k0 = 0
                for ci, w in enumerate(chunks):
                    t = sb.tile([P, w, width], dt, tag=f"{tag}{ci}",
                                name=f"{tag}{ci}")
                    ch.append((t, k0, w))
                    k0 += w
                return ch

            def xt_dma(dram, ch, ci, gp=False):
                t, k0, w = ch[ci]
                if gp:
                    nc.gpsimd.dma_start(t[:], dram[:, k0:k0 + w, :])
                else:
                    dma2(t[:], dram[:, k0:k0 + w, :])

            first8 = GU8[0]
            xt8_ch = xt16_ch = None
            wsel_sb = misc.tile([P, NV, CAP], F16, tag="wsel")
            if first8:
                xt8_ch = xt_alloc(F8, "xt8", CAP8, XT8_CH)
                for ci in range(len(XT8_CH)):
                    xt_dma(xt8, xt8_ch, ci)
                dma2(wsel_sb[:], wselv[:])
                if N16M:
                    xt16_ch = xt_alloc(F16, "xt16", CAP, XT16_CH)
                    for ci in range(len(XT16_CH)):
                        xt_dma(xt16, xt16_ch, ci)
            else:
                xt16_ch = xt_alloc(F16, "xt16", CAP, XT16_CH)
                for ci in range(len(XT16_CH)):
                    xt_dma(xt16, xt16_ch, ci)
                dma2(wsel_sb[:], wselv[:])
                if N8M:
                    # fp8 tokens are consumed late (fp8 m-tiles sit at the
                    # end): spread their DMAs across the m-loop on SWDGE
                    xt8_ch = xt_alloc(F8, "xt8", CAP8, XT8_CH)
            xt8_next = [0]

            hh8 = sb.tile([P, KD8N, CAP8], F8, name="hh8") if KD8N else None
            hh16 = sb.tile([P, KD16N, CAP], F16, name="hh16") if KD16N else None
            wd8_sb = sb.tile([P, KD8N, D], F8, tag="wd8_sb", name="wd8_sb") \
                if KD8N else None
            wd16_sb = sb.tile([P, KD16N, D], F16, tag="wd16_sb", name="wd16_sb") \
                if KD16N else None

            def find_ch(ch, k):
                return next((t, k0, w) for (t, k0, w) in ch if k0 <= k < k0 + w)

            # ---- gate/up projections + silu*mul*wsel, feature-major ----
            for m in range(KH):
                if m == 0:
                    wg_col, wu_col = wg_c0, wu_c0
                else:
                    wg_col, wu_col = w_cols(m)
                # stage the down-proj weights and late-consumed fp8 tokens
                # mid-stream, in quarter slices so no single piece
                # head-of-line-blocks the next m-tile's weight columns
                if 3 <= m <= 6 and KD8N:
                    q = m - 3
                    s0 = KD8N * q // 4
                    s1 = KD8N * (q + 1) // 4
                    if s1 > s0:
                        dma2(wd8_sb[:, s0:s1, :], wd8[:, s0:s1, :])
                if 3 <= m <= 6 and KD16N:
                    q = m - 3
                    s0 = KD16N * q // 4
                    s1 = KD16N * (q + 1) // 4
                    if s1 > s0:
                        dma2(wd16_sb[:, s0:s1, :], wd16[:, s0:s1, :])
                if not first8 and N8M:
                    # one chunk per iteration, all in place before the
                    # first fp8 m-tile needs them
                    while xt8_next[0] < len(XT8_CH) and (
                        xt8_next[0] <= m or m == M8[0] - 1
                    ):
                        xt_dma(xt8, xt8_ch, xt8_next[0])
                        xt8_next[0] += 1
                is8 = GU8[m]
                xch = xt8_ch if is8 else xt16_ch
                ps_g = [
                    pgu_pool.tile([P, 512], F32, tag="ps_g", name=f"ps_g{gi}", bufs=2)
                    for gi in range(len(GU_FULL))
                ]
                ps_u = [
                    pgu_pool.tile([P, 512], F32, tag="ps_u", name=f"ps_u{gi}", bufs=2)
                    for gi in range(len(GU_FULL))
                ]
                if GU_TAIL:
                    # full-bank tiles so these land in separate PSUM banks:
                    # interleaved accumulation groups sharing one bank
                    # corrupt each other (per-bank clear)
                    ps_tg = psum_pool.tile([P, 512], F32, tag="ps_tg", bufs=2)
                    ps_tu = psum_pool.tile([P, 512], F32, tag="ps_tu", bufs=2)
                # fp8 m-tiles: DoubleRow over 8 k-pairs for the 512-wide
                # groups; the 40-token tail group runs plain fp8 (DoubleRow
                # disables FWL and loses below FD~128). fp16 m-tiles: 16 ks.
                nsteps = KD // 2 if is8 else KD

                def _mm(which, gi, step):
                    w_col = wg_col if which == "g" else wu_col
                    if gi == "T":
                        dst = (ps_tg if which == "g" else ps_tu)[:, :GU_TAIL_N]
                        c0, cn = GU_TAIL[0], GU_TAIL_N
                        for k in ((2 * step, 2 * step + 1) if is8 else (step,)):
                            t, k0, _ = find_ch(xch, k)
                            nc.tensor.matmul(
                                dst,
                                lhsT=w_col[:, k, :],
                                rhs=t[:, k - k0, c0:c0 + cn],
                                start=(k == 0),
                                stop=(k == KD - 1),
                            )
                        return
                    dst = (ps_g if which == "g" else ps_u)[gi][:, :512]
                    c0, cn = GU_FULL[gi]
                    if is8:
                        k = 2 * step
                        t, k0, _ = find_ch(xch, k)
                        nc.tensor.matmul(
                            dst,
                            lhsT=w_col[:, k:k + 2, :],
                            rhs=t[:, k - k0:k - k0 + 2, c0:c0 + cn],
                            start=(step == 0),
                            stop=(step == nsteps - 1),
                            perf_mode=PM_DR,
                        )
                    else:
                        k = step
                        t, k0, _ = find_ch(xch, k)
                        nc.tensor.matmul(
                            dst,
                            lhsT=w_col[:, k, :],
                            rhs=t[:, k - k0, c0:c0 + cn],
                            start=(step == 0),
                            stop=(step == nsteps - 1),
                        )

                tail = [("g", "T"), ("u", "T")] if GU_TAIL else []
                for step in range(nsteps):
                    if step == 0:
                        # tail psum (single buffer) is still being read by
                        # the previous m's silu chain: issue its MMs last
                        order = [("g", 0), ("g", 1), ("u", 0), ("u", 1)] + tail
                    elif step == nsteps - 1:
                        # finish group 0 (g then u) first so its silu chain
                        # starts while the remaining matmuls stream
                        order = [("g", 0), ("u", 0), ("g", 1), ("u", 1)] + tail
                    else:
                        order = [("g", 0), ("g", 1)] + tail[:1] + \
                                [("u", 0), ("u", 1)] + tail[1:]
                    for which, gi in order:
                        _mm(which, gi, step)
                gu_parts = [
                    (c0, cn, ps_g[gi][:, :cn], ps_u[gi][:, :cn])
                    for gi, (c0, cn) in enumerate(GU_FULL)
                ]
                if GU_TAIL:
                    gu_parts.append((
                        GU_TAIL[0], GU_TAIL_N,
                        ps_tg[:, :GU_TAIL_N], ps_tu[:, :GU_TAIL_N],
                    ))
                dn8 = DN8[m // 2]
                hh_t, hh_s = (hh8, K8.index(m)) if dn8 else (hh16, K16.index(m))
                for c0, cn, pg_ap, pu_ap in gu_parts:
                    tmp = misc.tile([P, 512], F16, tag="silu", bufs=2)
                    nc.scalar.activation(
                        tmp[:, :cn], pg_ap, AF.Silu,
                        scale=(1.0 / SW) if is8 else 1.0,
                    )
                    tmp2 = misc.tile([P, 512], F16, tag="uw", bufs=2)
                    nc.vector.tensor_tensor(
                        tmp2[:, :cn],
                        pu_ap,
                        wsel_sb[:, VIDX[m], c0:c0 + cn],
                        op=OP.mult,
                    )
                    nc.vector.tensor_tensor(
                        hh_t[:, hh_s, c0:c0 + cn],
                        tmp[:, :cn],
                        tmp2[:, :cn],
                        op=OP.mult,
                    )

            # ---- down projection, token-major out [CAP, D] ----
            # ct outer / dgi inner; fp8 and fp16 k-pairs accumulate in
            # separate PSUM banks (different scales), combined in the
            # epilogue. o_row split into left/right half-tiles so the
            # left half DMAs out while the right half computes.
            P8 = list(range(0, KD8N, 2))
            P16 = list(range(0, KD16N, 2))
            for (t0, tn) in CT_TILES:
                o_half = [
                    misc.tile([P, D // 2], F16, tag=f"o{h}", name=f"o{h}", bufs=3)
                    for h in range(2)
                ]
                for dgi in range(D // DG):
                    if KD8N:
                        ps8 = ps_pool.tile([P, DG], F32, tag="ps_g", bufs=2)
                        for j, s in enumerate(P8):
                            nc.tensor.matmul(
                                ps8[:tn, :],
                                lhsT=hh8[:, s:s + 2, t0:t0 + tn],
                                rhs=wd8_sb[:, s:s + 2, dgi * DG:(dgi + 1) * DG],
                                start=(j == 0),
                                stop=(j == len(P8) - 1),
                                perf_mode=PM_DR,
                            )
                    if KD16N:
                        ps16 = ps_pool.tile([P, DG], F32, tag="ps_u", bufs=2)
                        for j, s in enumerate(P16):
                            for kk in (s, s + 1):
                                nc.tensor.matmul(
                                    ps16[:tn, :],
                                    lhsT=hh16[:, kk, t0:t0 + tn],
                                    rhs=wd16_sb[:, kk, dgi * DG:(dgi + 1) * DG],
                                    start=(kk == P16[0]),
                                    stop=(kk == P16[-1] + 1),
                                )
                    dst = o_half[dgi // 2][:tn, (dgi % 2) * DG:(dgi % 2 + 1) * DG]
                    if KD8N and KD16N:
                        nc.vector.scalar_tensor_tensor(
                            dst, ps8[:tn, :], 1.0 / (SH * SW), ps16[:tn, :],
                            op0=OP.mult, op1=OP.add,
                        )
                    elif KD8N:
                        nc.vector.tensor_scalar(
                            dst, ps8[:tn, :], 1.0 / (SH * SW), None, op0=OP.mult
                        )
                    else:
                        nc.vector.tensor_copy(dst, ps16[:tn, :])
                    if dgi % 2 == 1:
                        h = (tn + 1) // 2
                        o = o_half[dgi // 2]
                        dd = out[:, (dgi // 2) * 1024:(dgi // 2 + 1) * 1024]
                        nc.sync.dma_start(dd[t0:t0 + h, :], o[:h, :])
                        nc.scalar.dma_start(dd[t0 + h:t0 + tn, :], o[h:tn, :])
    nc.compile()
    return nc


def _feature_major(a2d, dtype=WNP):
    """[D, N] -> [P, D//P, N] (partition, k-tile, free), contiguous."""
    d, n = a2d.shape
    return np.ascontiguousarray(
        a2d.reshape(d // P, P, n).transpose(1, 0, 2).astype(dtype)
    )


def _host_expert(x_tok, wg_e, wu_e, wd_e):
    """Exact fp32 SwiGLU expert for rare capacity-overflow tokens."""
    g = x_tok @ wg_e
    u = x_tok @ wu_e
    hh = (g / (1.0 + np.exp(-g))) * u
    return hh @ wd_e


def kernel(hidden_states, W_gate, w_gate_proj, w_up_proj, w_down_proj):
    global _gate_nc, _moe_nc
    trace = os.environ.get("BASS_KERNEL_TRACE") == "1"

    hidden_states = np.asarray(hidden_states, dtype=np.float32)
    W_gate = np.asarray(W_gate, dtype=np.float32)
    w_gate_proj = np.asarray(w_gate_proj, dtype=np.float32)
    w_up_proj = np.asarray(w_up_proj, dtype=np.float32)
    w_down_proj = np.asarray(w_down_proj, dtype=np.float32)

    x = np.ascontiguousarray(hidden_states.reshape(T, D))
    xh = x.astype(WNP)
    x8 = x.astype(E4NP) if N8M else None

    if _gate_nc is None:
        _gate_nc = _build_gate_nc()
    if _moe_nc is None:
        _moe_nc = _build_moe_nc()

    # ---- phase 1: gate on device (data-parallel over tokens) ----
    wgt_host = _feature_major(W_gate.T.astype(WNP))  # [P, KD, E]
    in_maps1 = []
    for c in range(N_CORES):
        xs = xh[c * TPC:(c + 1) * TPC]            # [TPC, D]
        in_maps1.append({
            "xst": _feature_major(np.ascontiguousarray(xs.T)),  # [P, KD, TPC]
            "wgt": wgt_host,
        })
    res1 = _run_spmd(_gate_nc, in_maps1, trace, "gate")
    LAST_EXEC_NS["gate"] = res1.exec_time_ns
    w = np.concatenate([r["wout"] for r in res1.results], axis=0)  # [T, E]

    # ---- host dispatch: route tokens to expert cores ----
    in_maps2 = []
    idx_list = []
    overflow = []  # (expert, token idx array) handled exactly on host
    cfg_key = "".join("1" if b else "0" for b in GU8 + DN8)
    for e in range(E):
        idx = np.flatnonzero(w[:, e] > 0.0)
        if len(idx) > CAP:
            overflow.append((e, idx[CAP:]))
            idx = idx[:CAP]
        idx_list.append(idx)
        ne = len(idx)
        im = {}
        if N16M:
            xt16_h = np.zeros((P, KD, CAP), WNP)
            xt16_h[:, :, :ne] = _feature_major(np.ascontiguousarray(xh[idx].T))
            im["xt16"] = xt16_h
        if N8M:
            xt8_h = np.zeros((P, KD, CAP8), E4NP)
            xt8_h[:, :, :ne] = _feature_major(
                np.ascontiguousarray(x8[idx].T), dtype=E4NP
            )
            im["xt8"] = xt8_h
        wsv = np.zeros((NV, CAP), WNP)
        for vi, s in enumerate(VS):
            wsv[vi, :ne] = (w[idx, e] * s).astype(WNP)
        im["wselv"] = np.ascontiguousarray(
            np.broadcast_to(wsv, (P, NV, CAP))
        )
        ck = (
            e, cfg_key, w_gate_proj.ctypes.data, float(w_gate_proj[e, 0, 0]),
            float(w_up_proj[e, 1, 1]), float(w_down_proj[e, 2, 2]),
        )
        if ck not in _wprep_cache:
            cw = {}
            wgf = w_gate_proj[e].reshape(KD, P, KH, P).transpose(2, 1, 0, 3)
            wuf = w_up_proj[e].reshape(KD, P, KH, P).transpose(2, 1, 0, 3)
            wdf = w_down_proj[e].reshape(KH, P, D).transpose(1, 0, 2)
            if N16M:
                cw["wg16"] = np.ascontiguousarray(wgf[M16]).astype(WNP)
                cw["wu16"] = np.ascontiguousarray(wuf[M16]).astype(WNP)
            if N8M:
                cw["wg8"] = np.ascontiguousarray(wgf[M8] * SW).astype(E4NP)
                cw["wu8"] = np.ascontiguousarray(wuf[M8] * SW).astype(E4NP)
            if KD16N:
                cw["wd16"] = np.ascontiguousarray(wdf[:, K16, :]).astype(WNP)
            if KD8N:
                cw["wd8"] = np.ascontiguousarray(
                    wdf[:, K8, :] * SW
                ).astype(E4NP)
            _wprep_cache[ck] = cw
        im.update(_wprep_cache[ck])
        in_maps2.append(im)

    # ---- phase 2: expert FFN on device (expert-parallel) ----
    res2 = _run_spmd(_moe_nc, in_maps2, trace, "moe")
    LAST_EXEC_NS["moe"] = res2.exec_time_ns

    # ---- host combine: scatter-add + residual ----
    y = x.copy()
    for e in range(E):
        idx = idx_list[e]
        y[idx] += res2.results[e]["out"][:len(idx)].astype(np.float32)
    for e, idx in overflow:
        y[idx] += w[idx, e:e + 1] * _host_expert(
            x[idx], w_gate_proj[e], w_up_proj[e], w_down_proj[e]
        ).astype(np.float32)
    return y.reshape(B, S, D)


# revision 43
# speedup vs baseline: 1.0279x; 1.0279x over previous
"""DeepSeek-style MoE forward on 8 Trainium2 NeuronCores.

Strategy (expert-parallel, per sharding hint):
  Phase 1 (device, data-parallel): each core computes the softmax gate +
    top-2 renormalized weights for its 512-token slice, in f16 with a
    sigmoid-based renormalization (w1 = sigmoid(s1 - s2), exact).
  Host dispatch: tokens are routed to expert cores by the device-computed
    top-k weights (the "all-to-all", emulated with numpy gathers; layout
    transposed to feature-major for the device).
  Phase 2 (device, expert-parallel): core e holds expert e's weights and
    runs the SwiGLU FFN on its gathered tokens; the routing weight is
    folded into hh before the down projection. Rare capacity overflow
    falls back to exact host math.
  Host combine: scatter-add expert outputs + residual.

Mixed fp8 precision (the big lever beyond the 240us fp16 baseline):
  The tensor engine runs fp8e4 (e4m3) matmuls at 2x rate via DoubleRow
  perf mode (two 128-deep k-subtiles per instruction, 0.5 cyc/row).
  Full-fp8 blows the 2e-2 accuracy budget (sim: 2.7e-2), so precision is
  configurable per stage and spent where it buys the most cycles:
    BASS_GU8: 8 bits, m-th H-column-tile of the gate/up projections in
      fp8 (x8 @ 32*w8, psum descaled in the silu/mul epilogue).
    BASS_DN8: 4 bits, p-th k-pair of the down projection in fp8
      (hh stored as e4m3(8*hh*wsel), wd8 = e4m3(32*wd); fp8 and fp16
      k-pairs accumulate in separate PSUM banks, combined in the
      epilogue: o = ps16 + ps8/256).
  Error (sim, validated within 0.3% of device at all16): all16 3.0e-3,
  dn8x4 1.62e-2, dn8x4+gu1 1.80e-2, dn8x4+gu2 1.96e-2, full8 2.7e-2.

Perf notes inherited from the fp16 baseline (still load-bearing):
  - top-2 renorm via sigmoid(s1-s2) on logits, no exp/divide.
  - CAP 1064 (max expert load 1063); token-dim groups (512,512,40) for
    gate/up, tail groups in separate full PSUM banks (interleaved
    accumulation groups sharing a bank corrupt).
  - PE warm-up matmuls at kernel start flip the HAM clock gate to 8/8.
  - Every transfer split across both HWDGE queues by partition halves,
    issued in consumption order; weight-column pools 4-deep.
  - Down-proj emits full-row output DMAs per 128-token tile; o_row is
    split into left/right half-tiles so the first half DMAs out while
    the second half computes; narrow tail ct tile last.
  - Trace facts (fp16 baseline): tensor busy 188.7us = throttled
    roofline (throttle_avg_util_limit 91% -> eff ~2.19 GHz); ~27us idle
    split between launch preamble (~7us fixed), DMA cold-start gaps and
    end drain. exec_time_ns = last_useful - first_useful on core 0.

Self-contained: shapes hardcoded from the problem spec.
"""
import os
import sys

import numpy as np
import ml_dtypes

if "/opt/trn_rl_repo" not in sys.path:
    sys.path.insert(0, "/opt/trn_rl_repo")

import concourse.tile as tile
from concourse import bacc, mybir
from concourse.bass_utils import run_bass_kernel_spmd
from concourse.masks import make_identity

B, S, D, E, H = 2, 2048, 2048, 8, 1024
T = B * S            # 4096 tokens
N_CORES = 8
TPC = T // N_CORES   # 512 tokens/core for the gate phase
CAP = int(os.environ.get("BASS_MOE_CAP", "1064"))  # per-expert capacity
P = 128
KD = D // P          # 16
KH = H // P          # 8
GU_FULL = [(i * 512, 512) for i in range(CAP // 512)]
GU_TAIL = (CAP - CAP % 512, CAP % 512) if CAP % 512 else None
GU_TAIL_N = GU_TAIL[1] if GU_TAIL else 0
CT_TILES = []
_c0 = 0
while _c0 < CAP:
    CT_TILES.append((_c0, min(P, CAP - _c0)))
    _c0 += P
DG = 512             # down-proj free-dim group
F32 = mybir.dt.float32
F16 = mybir.dt.float16
F8 = mybir.dt.float8e4
WNP = np.float16
E4NP = ml_dtypes.float8_e4m3
AF = mybir.ActivationFunctionType
OP = mybir.AluOpType
AX = mybir.AxisListType
PM_DR = mybir.MatmulPerfMode.DoubleRow

# ---- mixed-precision config ----
GU8 = [c == "1" for c in os.environ.get("BASS_GU8", "00000001")]
DN8 = [c == "1" for c in os.environ.get("BASS_DN8", "1111")]
assert len(GU8) == KH and len(DN8) == KH // 2
SW = 32.0            # fp8 weight scale (w*SW quantized to e4m3)
SH = 8.0             # fp8 hh scale
# DoubleRow LDWEIGHTS requires the stationary k-pair dim step % 16 == 0:
# fp8 token-major tiles are padded from CAP to CAP8
CAP8 = ((CAP + 15) // 16) * 16
K8 = [k for p in range(4) if DN8[p] for k in (2 * p, 2 * p + 1)]
K16 = [k for p in range(4) if not DN8[p] for k in (2 * p, 2 * p + 1)]
KD8N, KD16N = len(K8), len(K16)
M8 = [m for m in range(KH) if GU8[m]]
M16 = [m for m in range(KH) if not GU8[m]]
N8M, N16M = len(M8), len(M16)
M8_IDX = {m: i for i, m in enumerate(M8)}
M16_IDX = {m: i for i, m in enumerate(M16)}
# wsel variant per m: u-epilogue multiplier (SH if down-fp8) / (SW if gu-fp8)
_MSCALE = [
    (SH if DN8[m // 2] else 1.0) / (SW if GU8[m] else 1.0) for m in range(KH)
]
VS = sorted(set(_MSCALE))
NV = len(VS)
VIDX = [VS.index(s) for s in _MSCALE]
# xt chunking in k-tiles; fp8 chunks must have even widths/starts so
# DoubleRow k-pairs never straddle a chunk boundary. Pieces are sized
# ~0.5MB: per-DMA completion latency (~1.5us) bounds queue throughput,
# so fewer/bigger transfers beat fine-grained streaming.
XT16_CH = [2, 2, 4, 4, 4]
XT8_CH = [2, 2, 4, 4, 4]

_gate_nc = None
_moe_nc = None
_wprep_cache = {}
LAST_EXEC_NS = {"gate": None, "moe": None}
_TMPDIR = os.environ.get("BASS_KERNEL_TMPDIR")


def _axon_reset():
    """Recover a wedged NeuronCore via the axon client's reset entry
    point. Best-effort."""
    try:
        import ctypes

        lib = ctypes.CDLL("/opt/axon/libaxon_pjrt.so")
        lib.axon_reset.restype = ctypes.c_int64
        lib.axon_reset()
    except Exception:
        pass


_run_counter = [0]


def _run_spmd(nc, in_maps, trace, tag):
    _run_counter[0] += 1
    tag = f"{tag}_{_run_counter[0]}"
    try:
        return run_bass_kernel_spmd(
            nc, in_maps, core_ids=list(range(N_CORES)), trace=trace,
            tmpdir=(_TMPDIR + "/" + tag) if (trace and _TMPDIR) else None,
        )
    except Exception:
        _axon_reset()
        return run_bass_kernel_spmd(
            nc, in_maps, core_ids=list(range(N_CORES)), trace=trace,
            tmpdir=(_TMPDIR + "/" + tag + "_retry") if (trace and _TMPDIR) else None,
        )


def _emit_warmup(nc, sbuf_pool, psum_pool, psum_tag, n_mm, width, bufs=None,
                 act_funcs=(), mm_width=None):
    """Dummy matmuls with no DMA deps: keep the PE busy from the moment its
    preamble barrier clears so the HAM clock gate flips to 8/8 before the
    first real matmul. Also preloads activation tables (act_funcs) so the
    ~1.3us ACT_TABLE_LOAD overlaps the input DMA window instead of
    stalling the first real activation."""
    warm_src = sbuf_pool.tile([P, width], F16, tag="warm_src")
    nc.gpsimd.memset(warm_src[:], 0.0)
    if act_funcs:
        # separate tile: the act-table preload must not add a dependency
        # onto the warm-up matmuls' source
        warm_act = sbuf_pool.tile([1, 2], F32, tag="warm_act")
        nc.gpsimd.memset(warm_act[:], 0.0)
        for fn in act_funcs:
            nc.scalar.activation(warm_act[:, :1], warm_act[:, 1:], fn)
    mw = mm_width or width
    ps = psum_pool.tile([P, width], F32, tag=psum_tag, name="warm_ps", bufs=bufs)
    for _ in range(n_mm):
        nc.tensor.matmul(
            ps[:, :mw], lhsT=warm_src[:, :P], rhs=warm_src[:, :mw],
            start=True, stop=True,
        )


def _build_gate_nc():
    """Gate kernel: per-core 512-token slice -> renormalized top-2 weights.

    Inputs (feature-major, host-transposed f16 layout):
      xst  [P, KD, TPC]  slice of x^T   (xst[p, k, t] = x[t, k*P+p])
      wgt  [P, KD, E]    W_gate^T      (wgt[p, k, e] = W_gate[e, k*P+p])
    Output:
      wout [TPC, E]  w[t, e] = renormalized top-2 weight, 0 if not selected

    scores^T = wgt.T @ x^T with the 8-column gate weight stationary, then
    PE-transposed back to token-major. Top-2 renormalization uses
    w1 = sigmoid(s1 - s2), w2 = sigmoid(s2 - s1) (the softmax denominator
    cancels), batched across all 4 token tiles in an 11-op chain.
    """
    nc = bacc.Bacc(None, target_bir_lowering=False, enable_partition_id=False)
    xst = nc.dram_tensor("xst", [P, KD, TPC], F16, kind="ExternalInput")
    wgt = nc.dram_tensor("wgt", [P, KD, E], F16, kind="ExternalInput")
    wout = nc.dram_tensor("wout", [TPC, E], F32, kind="ExternalOutput")
    NTT = TPC // P  # 4 token tiles

    with tile.TileContext(nc) as tc:
        with (
            tc.tile_pool(name="xp", bufs=1) as xp,
            tc.tile_pool(name="wp", bufs=1) as wp,
            tc.tile_pool(name="psum", bufs=2, space="PSUM") as psum_pool,
            tc.tile_pool(name="v", bufs=2) as vp,
        ):
            _emit_warmup(nc, wp, psum_pool, "warm", 5, 256,
                         act_funcs=(AF.Sigmoid,))
            ident = wp.tile([P, P], F32)
            make_identity(nc, ident[:])
            wgt_sb = wp.tile([P, KD, E], F16)
            nc.sync.dma_start(wgt_sb[:], wgt[:])
            HP = P // 2
            # xst in 8 chunks of 2 k-tiles, each split across both HWDGE
            # queues by partition halves (disjoint SDMA engine sets);
            # small chunks keep the matmul stream fed from the first one
            CH = [2] * 8
            xst_ch = []
            k0 = 0
            for ci, w in enumerate(CH):
                t = xp.tile([P, w, TPC], F16, tag=f"xst{ci}", name=f"xst{ci}")
                s = xst[:, k0:k0 + w, :]
                nc.sync.dma_start(t[0:HP], s[0:HP])
                nc.scalar.dma_start(t[HP:P], s[HP:P])
                xst_ch.append((t, k0, w))
                k0 += w

            # scores^T [E, TPC], contraction over D in 16 k-tiles
            ps_st = psum_pool.tile([E, TPC], F32, tag="ps_st")
            for k in range(KD):
                t, k0, w = next(c for c in xst_ch if c[1] <= k < c[1] + c[2])
                nc.tensor.matmul(
                    ps_st[:],
                    lhsT=wgt_sb[:, k, :],
                    rhs=t[:, k - k0, :],
                    start=(k == 0),
                    stop=(k == KD - 1),
                )
            st_sb = vp.tile([E, TPC], F32, tag="st")
            nc.vector.tensor_copy(st_sb[:], ps_st[:])

            # transpose back to token-major: sc [P, NTT, E]
            sc = vp.tile([P, NTT, E], F32, tag="sc")
            for tt in range(NTT):
                ps = psum_pool.tile([P, E], F32, tag="scores")
                nc.tensor.transpose(
                    ps[:], st_sb[:, tt * P:(tt + 1) * P], ident[:E, :E]
                )
                nc.vector.tensor_copy(sc[:, tt, :], ps[:])

            # batched top-2 chain over [P, NTT, E]
            m1 = vp.tile([P, NTT, 1], F32, tag="m1")
            nc.vector.tensor_reduce(m1[:], sc[:], op=OP.max, axis=AX.X)
            mask1 = vp.tile([P, NTT, E], F32, tag="mask1")
            nc.vector.tensor_tensor(
                mask1[:], sc[:], m1[:].broadcast_to([P, NTT, E]), op=OP.is_equal
            )
            # sc2 = sc - 1e30*mask1 (knock out the max) in one op
            sc2 = vp.tile([P, NTT, E], F32, tag="sc2")
            nc.vector.scalar_tensor_tensor(
                sc2[:], mask1[:], -1e30, sc[:], op0=OP.mult, op1=OP.add
            )
            m2 = vp.tile([P, NTT, 1], F32, tag="m2")
            nc.vector.tensor_reduce(m2[:], sc2[:], op=OP.max, axis=AX.X)
            mask2 = vp.tile([P, NTT, E], F32, tag="mask2")
            nc.vector.tensor_tensor(
                mask2[:], sc2[:], m2[:].broadcast_to([P, NTT, E]), op=OP.is_equal
            )
            d12 = vp.tile([P, NTT, 1], F32, tag="d12")
            nc.vector.tensor_tensor(d12[:], m1[:], m2[:], op=OP.subtract)
            w1 = vp.tile([P, NTT, 1], F32, tag="w1")
            nc.scalar.activation(w1[:], d12[:], AF.Sigmoid)
            w2 = vp.tile([P, NTT, 1], F32, tag="w2")
            nc.scalar.activation(w2[:], d12[:], AF.Sigmoid, scale=-1.0)
            o1 = vp.tile([P, NTT, E], F32, tag="o1")
            nc.vector.tensor_tensor(
                o1[:], mask1[:], w1[:].broadcast_to([P, NTT, E]), op=OP.mult
            )
            w_all = vp.tile([P, NTT, E], F32, tag="w_all")
            nc.vector.tensor_tensor(
                w_all[:], mask2[:], w2[:].broadcast_to([P, NTT, E]), op=OP.mult
            )
            nc.vector.tensor_tensor(w_all[:], w_all[:], o1[:], op=OP.add)
            nc.scalar.dma_start(
                wout.rearrange("(tt p) e -> p tt e", p=P), w_all[:]
            )
    nc.compile()
    return nc


def _build_moe_nc():
    """Expert FFN kernel: out[c, :] = (silu(x_c @ Wg) * (x_c @ Wu) * wsel[c]) @ Wd.

    Inputs (host-prepared feature/contraction-major layouts; fp8 weights
    pre-scaled by SW, descaled on device):
      xt16   [P, KD, CAP]      f16  gathered tokens (if any fp16 m-tile)
      xt8    [P, KD, CAP]      f8   e4m3(x) (if any fp8 m-tile)
      wg16   [N16M, P, KD, P]  f16  gate-proj columns for fp16 m-tiles
      wg8    [N8M, P, KD, P]   f8   e4m3(SW*w) for fp8 m-tiles
      wu16/wu8                 likewise for up-proj
      wd16   [P, KD16N, D]     f16  down-proj k-slices for fp16 pairs
      wd8    [P, KD8N, D]      f8   e4m3(SW*wd) for fp8 pairs
      wselv  [P, NV, CAP]      f16  routing weight * per-m epilogue scale
    Output:
      out    [CAP, D] f16
    """
    nc = bacc.Bacc(None, target_bir_lowering=False, enable_partition_id=False)
    xt16 = nc.dram_tensor("xt16", [P, KD, CAP], F16, kind="ExternalInput") \
        if N16M else None
    xt8 = nc.dram_tensor("xt8", [P, KD, CAP8], F8, kind="ExternalInput") \
        if N8M else None
    # gate+up weight columns stacked per m-tile: one ~1MB DMA pair per m
    wgu16 = nc.dram_tensor("wgu16", [N16M, P, 2, KD, P], F16,
                           kind="ExternalInput") if N16M else None
    wgu8 = nc.dram_tensor("wgu8", [N8M, P, 2, KD, P], F8,
                          kind="ExternalInput") if N8M else None
    wd16 = nc.dram_tensor("wd16", [P, KD16N, D], F16, kind="ExternalInput") \
        if KD16N else None
    wd8 = nc.dram_tensor("wd8", [P, KD8N, D], F8, kind="ExternalInput") \
        if KD8N else None
    wselv = nc.dram_tensor("wselv", [P, NV, CAP], F16, kind="ExternalInput")
    out = nc.dram_tensor("out", [CAP, D], F16, kind="ExternalOutput")

    with tile.TileContext(nc) as tc:
        with (
            tc.tile_pool(name="sb", bufs=1) as sb,
            tc.tile_pool(name="ps", bufs=1, space="PSUM") as ps_pool,
        ):
            misc = sb
            psum_pool = pgu_pool = ps_pool
            # warm-up psum shares the ps_tg tag: all 8 PSUM banks go to
            # ps_g/ps_u/ps_tg/ps_tu at bufs=2 (tail groups double-buffered
            # kills the ~0.8us stall at each m-tile boundary)
            _emit_warmup(nc, misc, ps_pool, "ps_tg", 10, 512, bufs=2,
                         act_funcs=(AF.Silu,), mm_width=256)

            HP = P // 2

            def dma2(dst, src):
                nc.sync.dma_start(dst[0:HP], src[0:HP])
                nc.scalar.dma_start(dst[HP:P], src[HP:P])

            def w_cols(m):
                if GU8[m]:
                    src, dt8, tag = wgu8[M8_IDX[m]], F8, "8"
                else:
                    src, dt8, tag = wgu16[M16_IDX[m]], F16, "16"
                t = sb.tile([P, 2, KD, P], dt8, tag=f"wcol{tag}",
                            name=f"wcol{tag}_{m}", bufs=4)
                dma2(t[:], src)
                return t[:, 0], t[:, 1]

            wg_c0, wu_c0 = w_cols(0)

            # xt chunk tiles; DMAs are placed individually: early chunks on
            # the two HWDGE queues in consumption order, the big tail
            # chunks and the late-consumed fp8 tokens on the gpsimd SWDGE
            # queue (3rd issue stream; all three share the 16 SDMA engines
            # but issue windows no longer serialize)
            def xt_alloc(dt, tag, width, chunks):
                ch = []
                k0 = 0
                for ci, w in enumerate(chunks):
                    t = sb.tile([P, w, width], dt, tag=f"{tag}{ci}",
                                name=f"{tag}{ci}")
                    ch.append((t, k0, w))
                    k0 += w
                return ch

            def xt_dma(dram, ch, ci, gp=False):
                t, k0, w = ch[ci]
                if gp:
                    nc.gpsimd.dma_start(t[:], dram[:, k0:k0 + w, :])
                else:
                    dma2(t[:], dram[:, k0:k0 + w, :])

            first8 = GU8[0]
            xt8_ch = xt16_ch = None
            wsel_sb = misc.tile([P, NV, CAP], F16, tag="wsel")
            if first8:
                xt8_ch = xt_alloc(F8, "xt8", CAP8, XT8_CH)
                for ci in range(len(XT8_CH)):
                    xt_dma(xt8, xt8_ch, ci)
                dma2(wsel_sb[:], wselv[:])
                if N16M:
                    xt16_ch = xt_alloc(F16, "xt16", CAP, XT16_CH)
                    for ci in range(len(XT16_CH)):
                        xt_dma(xt16, xt16_ch, ci)
            else:
                xt16_ch = xt_alloc(F16, "xt16", CAP, XT16_CH)
                for ci in range(len(XT16_CH)):
                    xt_dma(xt16, xt16_ch, ci)
                dma2(wsel_sb[:], wselv[:])
                if N8M:
                    # fp8 tokens are consumed late (fp8 m-tiles sit at the
                    # end): spread their DMAs across the m-loop on SWDGE
                    xt8_ch = xt_alloc(F8, "xt8", CAP8, XT8_CH)
            xt8_next = [0]

            hh8 = sb.tile([P, KD8N, CAP8], F8, name="hh8") if KD8N else None
            hh16 = sb.tile([P, KD16N, CAP], F16, name="hh16") if KD16N else None
            wd8_sb = sb.tile([P, KD8N, D], F8, tag="wd8_sb", name="wd8_sb") \
                if KD8N else None
            wd16_sb = sb.tile([P, KD16N, D], F16, tag="wd16_sb", name="wd16_sb") \
                if KD16N else None

            def find_ch(ch, k):
                return next((t, k0, w) for (t, k0, w) in ch if k0 <= k < k0 + w)

            # ---- gate/up projections + silu*mul*wsel, feature-major ----
            for m in range(KH):
                if m == 0:
                    wg_col, wu_col = wg_c0, wu_c0
                else:
                    wg_col, wu_col = w_cols(m)
                # stage the down-proj weights and late-consumed fp8 tokens
                # mid-stream, in quarter slices so no single piece
                # head-of-line-blocks the next m-tile's weight columns
                if 3 <= m <= 6 and KD8N:
                    q = m - 3
                    s0 = KD8N * q // 4
                    s1 = KD8N * (q + 1) // 4
                    if s1 > s0:
                        dma2(wd8_sb[:, s0:s1, :], wd8[:, s0:s1, :])
                if 3 <= m <= 6 and KD16N:
                    q = m - 3
                    s0 = KD16N * q // 4
                    s1 = KD16N * (q + 1) // 4
                    if s1 > s0:
                        dma2(wd16_sb[:, s0:s1, :], wd16[:, s0:s1, :])
                if not first8 and N8M:
                    # one chunk per iteration, all in place before the
                    # first fp8 m-tile needs them
                    while xt8_next[0] < len(XT8_CH) and (
                        xt8_next[0] <= m or m == M8[0] - 1
                    ):
                        xt_dma(xt8, xt8_ch, xt8_next[0])
                        xt8_next[0] += 1
                is8 = GU8[m]
                xch = xt8_ch if is8 else xt16_ch
                ps_g = [
                    pgu_pool.tile([P, 512], F32, tag="ps_g", name=f"ps_g{gi}", bufs=2)
                    for gi in range(len(GU_FULL))
                ]
                ps_u = [
                    pgu_pool.tile([P, 512], F32, tag="ps_u", name=f"ps_u{gi}", bufs=2)
                    for gi in range(len(GU_FULL))
                ]
                if GU_TAIL:
                    # full-bank tiles so these land in separate PSUM banks:
                    # interleaved accumulation groups sharing one bank
                    # corrupt each other (per-bank clear)
                    ps_tg = psum_pool.tile([P, 512], F32, tag="ps_tg", bufs=2)
                    ps_tu = psum_pool.tile([P, 512], F32, tag="ps_tu", bufs=2)
                # fp8 m-tiles: DoubleRow over 8 k-pairs for the 512-wide
                # groups; the 40-token tail group runs plain fp8 (DoubleRow
                # disables FWL and loses below FD~128). fp16 m-tiles: 16 ks.
                nsteps = KD // 2 if is8 else KD

                def _mm(which, gi, step):
                    w_col = wg_col if which == "g" else wu_col
                    if gi == "T":
                        dst = (ps_tg if which == "g" else ps_tu)[:, :GU_TAIL_N]
                        c0, cn = GU_TAIL[0], GU_TAIL_N
                        for k in ((2 * step, 2 * step + 1) if is8 else (step,)):
                            t, k0, _ = find_ch(xch, k)
                            nc.tensor.matmul(
                                dst,
                                lhsT=w_col[:, k, :],
                                rhs=t[:, k - k0, c0:c0 + cn],
                                start=(k == 0),
                                stop=(k == KD - 1),
                            )
                        return
                    dst = (ps_g if which == "g" else ps_u)[gi][:, :512]
                    c0, cn = GU_FULL[gi]
                    if is8:
                        k = 2 * step
                        t, k0, _ = find_ch(xch, k)
                        nc.tensor.matmul(
                            dst,
                            lhsT=w_col[:, k:k + 2, :],
                            rhs=t[:, k - k0:k - k0 + 2, c0:c0 + cn],
                            start=(step == 0),
                            stop=(step == nsteps - 1),
                            perf_mode=PM_DR,
                        )
                    else:
                        k = step
                        t, k0, _ = find_ch(xch, k)
                        nc.tensor.matmul(
                            dst,
                            lhsT=w_col[:, k, :],
                            rhs=t[:, k - k0, c0:c0 + cn],
                            start=(step == 0),
                            stop=(step == nsteps - 1),
                        )

                tail = [("g", "T"), ("u", "T")] if GU_TAIL else []
                for step in range(nsteps):
                    if step == 0:
                        # tail psum (single buffer) is still being read by
                        # the previous m's silu chain: issue its MMs last
                        order = [("g", 0), ("g", 1), ("u", 0), ("u", 1)] + tail
                    elif step == nsteps - 1:
                        # finish group 0 (g then u) first so its silu chain
                        # starts while the remaining matmuls stream
                        order = [("g", 0), ("u", 0), ("g", 1), ("u", 1)] + tail
                    else:
                        order = [("g", 0), ("g", 1)] + tail[:1] + \
                                [("u", 0), ("u", 1)] + tail[1:]
                    for which, gi in order:
                        _mm(which, gi, step)
                gu_parts = [
                    (c0, cn, ps_g[gi][:, :cn], ps_u[gi][:, :cn])
                    for gi, (c0, cn) in enumerate(GU_FULL)
                ]
                if GU_TAIL:
                    gu_parts.append((
                        GU_TAIL[0], GU_TAIL_N,
                        ps_tg[:, :GU_TAIL_N], ps_tu[:, :GU_TAIL_N],
                    ))
                dn8 = DN8[m // 2]
                hh_t, hh_s = (hh8, K8.index(m)) if dn8 else (hh16, K16.index(m))
                for c0, cn, pg_ap, pu_ap in gu_parts:
                    tmp = misc.tile([P, 512], F16, tag="silu", bufs=2)
                    nc.scalar.activation(
                        tmp[:, :cn], pg_ap, AF.Silu,
                        scale=(1.0 / SW) if is8 else 1.0,
                    )
                    tmp2 = misc.tile([P, 512], F16, tag="uw", bufs=2)
                    nc.vector.tensor_tensor(
                        tmp2[:, :cn],
                        pu_ap,
                        wsel_sb[:, VIDX[m], c0:c0 + cn],
                        op=OP.mult,
                    )
                    nc.vector.tensor_tensor(
                        hh_t[:, hh_s, c0:c0 + cn],
                        tmp[:, :cn],
                        tmp2[:, :cn],
                        op=OP.mult,
                    )

            # ---- down projection, token-major out [CAP, D] ----
            # ct outer / dgi inner; fp8 and fp16 k-pairs accumulate in
            # separate PSUM banks (different scales), combined in the
            # epilogue. o_row split into left/right half-tiles so the
            # left half DMAs out while the right half computes.
            P8 = list(range(0, KD8N, 2))
            P16 = list(range(0, KD16N, 2))
            for (t0, tn) in CT_TILES:
                o_half = [
                    misc.tile([P, D // 2], F16, tag=f"o{h}", name=f"o{h}", bufs=3)
                    for h in range(2)
                ]
                for dgi in range(D // DG):
                    if KD8N:
                        ps8 = ps_pool.tile([P, DG], F32, tag="ps_g", bufs=2)
                        for j, s in enumerate(P8):
                            nc.tensor.matmul(
                                ps8[:tn, :],
                                lhsT=hh8[:, s:s + 2, t0:t0 + tn],
                                rhs=wd8_sb[:, s:s + 2, dgi * DG:(dgi + 1) * DG],
                                start=(j == 0),
                                stop=(j == len(P8) - 1),
                                perf_mode=PM_DR,
                            )
                    if KD16N:
                        ps16 = ps_pool.tile([P, DG], F32, tag="ps_u", bufs=2)
                        for j, s in enumerate(P16):
                            for kk in (s, s + 1):
                                nc.tensor.matmul(
                                    ps16[:tn, :],
                                    lhsT=hh16[:, kk, t0:t0 + tn],
                                    rhs=wd16_sb[:, kk, dgi * DG:(dgi + 1) * DG],
                                    start=(kk == P16[0]),
                                    stop=(kk == P16[-1] + 1),
                                )
                    dst = o_half[dgi // 2][:tn, (dgi % 2) * DG:(dgi % 2 + 1) * DG]
                    if KD8N and KD16N:
                        nc.vector.scalar_tensor_tensor(
                            dst, ps8[:tn, :], 1.0 / (SH * SW), ps16[:tn, :],
                            op0=OP.mult, op1=OP.add,
                        )
                    elif KD8N:
                        nc.vector.tensor_scalar(
                            dst, ps8[:tn, :], 1.0 / (SH * SW), None, op0=OP.mult
                        )
                    else:
                        nc.vector.tensor_copy(dst, ps16[:tn, :])
                    if dgi % 2 == 1:
                        h = (tn + 1) // 2
                        o = o_half[dgi // 2]
                        dd = out[:, (dgi // 2) * 1024:(dgi // 2 + 1) * 1024]
                        nc.sync.dma_start(dd[t0:t0 + h, :], o[:h, :])
                        nc.scalar.dma_start(dd[t0 + h:t0 + tn, :], o[h:tn, :])
    nc.compile()
    return nc


def _feature_major(a2d, dtype=WNP):
    """[D, N] -> [P, D//P, N] (partition, k-tile, free), contiguous."""
    d, n = a2d.shape
    return np.ascontiguousarray(
        a2d.reshape(d // P, P, n).transpose(1, 0, 2).astype(dtype)
    )


def _host_expert(x_tok, wg_e, wu_e, wd_e):
    """Exact fp32 SwiGLU expert for rare capacity-overflow tokens."""
    g = x_tok @ wg_e
    u = x_tok @ wu_e
    hh = (g / (1.0 + np.exp(-g))) * u
    return hh @ wd_e


def kernel(hidden_states, W_gate, w_gate_proj, w_up_proj, w_down_proj):
    global _gate_nc, _moe_nc
    trace = os.environ.get("BASS_KERNEL_TRACE") == "1"

    hidden_states = np.asarray(hidden_states, dtype=np.float32)
    W_gate = np.asarray(W_gate, dtype=np.float32)
    w_gate_proj = np.asarray(w_gate_proj, dtype=np.float32)
    w_up_proj = np.asarray(w_up_proj, dtype=np.float32)
    w_down_proj = np.asarray(w_down_proj, dtype=np.float32)

    x = np.ascontiguousarray(hidden_states.reshape(T, D))
    xh = x.astype(WNP)
    x8 = x.astype(E4NP) if N8M else None

    if _gate_nc is None:
        _gate_nc = _build_gate_nc()
    if _moe_nc is None:
        _moe_nc = _build_moe_nc()

    # ---- phase 1: gate on device (data-parallel over tokens) ----
    wgt_host = _feature_major(W_gate.T.astype(WNP))  # [P, KD, E]
    in_maps1 = []
    for c in range(N_CORES):
        xs = xh[c * TPC:(c + 1) * TPC]            # [TPC, D]
        in_maps1.append({
            "xst": _feature_major(np.ascontiguousarray(xs.T)),  # [P, KD, TPC]
            "wgt": wgt_host,
        })
    res1 = _run_spmd(_gate_nc, in_maps1, trace, "gate")
    LAST_EXEC_NS["gate"] = res1.exec_time_ns
    w = np.concatenate([r["wout"] for r in res1.results], axis=0)  # [T, E]

    # ---- host dispatch: route tokens to expert cores ----
    in_maps2 = []
    idx_list = []
    overflow = []  # (expert, token idx array) handled exactly on host
    cfg_key = "".join("1" if b else "0" for b in GU8 + DN8)
    for e in range(E):
        idx = np.flatnonzero(w[:, e] > 0.0)
        if len(idx) > CAP:
            overflow.append((e, idx[CAP:]))
            idx = idx[:CAP]
        idx_list.append(idx)
        ne = len(idx)
        im = {}
        if N16M:
            xt16_h = np.zeros((P, KD, CAP), WNP)
            xt16_h[:, :, :ne] = _feature_major(np.ascontiguousarray(xh[idx].T))
            im["xt16"] = xt16_h
        if N8M:
            xt8_h = np.zeros((P, KD, CAP8), E4NP)
            xt8_h[:, :, :ne] = _feature_major(
                np.ascontiguousarray(x8[idx].T), dtype=E4NP
            )
            im["xt8"] = xt8_h
        wsv = np.zeros((NV, CAP), WNP)
        for vi, s in enumerate(VS):
            wsv[vi, :ne] = (w[idx, e] * s).astype(WNP)
        im["wselv"] = np.ascontiguousarray(
            np.broadcast_to(wsv, (P, NV, CAP))
        )
        ck = (
            e, cfg_key, w_gate_proj.ctypes.data, float(w_gate_proj[e, 0, 0]),
            float(w_up_proj[e, 1, 1]), float(w_down_proj[e, 2, 2]),
        )
        if ck not in _wprep_cache:
            cw = {}
            wgf = w_gate_proj[e].reshape(KD, P, KH, P).transpose(2, 1, 0, 3)
            wuf = w_up_proj[e].reshape(KD, P, KH, P).transpose(2, 1, 0, 3)
            wdf = w_down_proj[e].reshape(KH, P, D).transpose(1, 0, 2)
            if N16M:
                # [N, 2, P, KD, P] -> [N, P, 2, KD, P]
                cw["wgu16"] = np.ascontiguousarray(
                    np.stack([wgf[M16], wuf[M16]], axis=1).transpose(0, 2, 1, 3, 4)
                ).astype(WNP)
            if N8M:
                cw["wgu8"] = np.ascontiguousarray(
                    np.stack([wgf[M8] * SW, wuf[M8] * SW], axis=1)
                    .transpose(0, 2, 1, 3, 4)
                ).astype(E4NP)
            if KD16N:
                cw["wd16"] = np.ascontiguousarray(wdf[:, K16, :]).astype(WNP)
            if KD8N:
                cw["wd8"] = np.ascontiguousarray(
                    wdf[:, K8, :] * SW
                ).astype(E4NP)
            _wprep_cache[ck] = cw
        im.update(_wprep_cache[ck])
        in_maps2.append(im)

    # ---- phase 2: expert FFN on device (expert-parallel) ----
    res2 = _run_spmd(_moe_nc, in_maps2, trace, "moe")
    LAST_EXEC_NS["moe"] = res2.exec_time_ns

    # ---- host combine: scatter-add + residual ----
    y = x.copy()
    for e in range(E):
        idx = idx_list[e]
        y[idx] += res2.results[e]["out"][:len(idx)].astype(np.float32)
    for e, idx in overflow:
        y[idx] += w[idx, e:e + 1] * _host_expert(
            x[idx], w_gate_proj[e], w_up_proj[e], w_down_proj[e]
        ).astype(np.float32)
    return y.reshape(B, S, D)


# revision 49
# speedup vs baseline: 1.0758x; 1.0466x over previous
"""DeepSeek-style MoE forward on 8 Trainium2 NeuronCores.

Strategy (expert-parallel, per sharding hint):
  Phase 1 (device, data-parallel): each core computes the softmax gate +
    top-2 renormalized weights for its 512-token slice, in f16 with a
    sigmoid-based renormalization (w1 = sigmoid(s1 - s2), exact).
  Host dispatch: tokens are routed to expert cores by the device-computed
    top-k weights (the "all-to-all", emulated with numpy gathers; layout
    transposed to feature-major for the device).
  Phase 2 (device, expert-parallel): core e holds expert e's weights and
    runs the SwiGLU FFN on its gathered tokens; the routing weight is
    folded into hh before the down projection. Rare capacity overflow
    falls back to exact host math.
  Host combine: scatter-add expert outputs + residual.

Mixed fp8 precision (the big lever beyond the 240us fp16 baseline):
  The tensor engine runs fp8e4 (e4m3) matmuls at 2x rate via DoubleRow
  perf mode (two 128-deep k-subtiles per instruction, 0.5 cyc/row).
  Full-fp8 blows the 2e-2 accuracy budget (sim: 2.7e-2), so precision is
  configurable per stage and spent where it buys the most cycles:
    BASS_GU8: 8 bits, m-th H-column-tile of the gate/up projections in
      fp8 (x8 @ 32*w8, psum descaled in the silu/mul epilogue).
    BASS_DN8: 4 bits, p-th k-pair of the down projection in fp8
      (hh stored as e4m3(8*hh*wsel), wd8 = e4m3(32*wd); fp8 and fp16
      k-pairs accumulate in separate PSUM banks, combined in the
      epilogue: o = ps16 + ps8/256).
  Error (sim, validated within 0.3% of device at all16): all16 3.0e-3,
  dn8x4 1.62e-2, dn8x4+gu1 1.80e-2, dn8x4+gu2 1.96e-2, full8 2.7e-2.

Perf notes inherited from the fp16 baseline (still load-bearing):
  - top-2 renorm via sigmoid(s1-s2) on logits, no exp/divide.
  - CAP 1064 (max expert load 1063); token-dim groups (512,512,40) for
    gate/up, tail groups in separate full PSUM banks (interleaved
    accumulation groups sharing a bank corrupt).
  - PE warm-up matmuls at kernel start flip the HAM clock gate to 8/8.
  - Every transfer split across both HWDGE queues by partition halves,
    issued in consumption order; weight-column pools 4-deep.
  - Down-proj emits full-row output DMAs per 128-token tile; o_row is
    split into left/right half-tiles so the first half DMAs out while
    the second half computes; narrow tail ct tile last.
  - Trace facts (fp16 baseline): tensor busy 188.7us = throttled
    roofline (throttle_avg_util_limit 91% -> eff ~2.19 GHz); ~27us idle
    split between launch preamble (~7us fixed), DMA cold-start gaps and
    end drain. exec_time_ns = last_useful - first_useful on core 0.

Self-contained: shapes hardcoded from the problem spec.
"""
import os
import sys

import numpy as np
import ml_dtypes

if "/opt/trn_rl_repo" not in sys.path:
    sys.path.insert(0, "/opt/trn_rl_repo")

import concourse.tile as tile
from concourse import bacc, mybir
from concourse.bass_utils import run_bass_kernel_spmd
from concourse.masks import make_identity

B, S, D, E, H = 2, 2048, 2048, 8, 1024
T = B * S            # 4096 tokens
N_CORES = 8
TPC = T // N_CORES   # 512 tokens/core for the gate phase
CAP = int(os.environ.get("BASS_MOE_CAP", "1064"))  # per-expert capacity
P = 128
KD = D // P          # 16
KH = H // P          # 8
GU_FULL = [(i * 512, 512) for i in range(CAP // 512)]
GU_TAIL = (CAP - CAP % 512, CAP % 512) if CAP % 512 else None
GU_TAIL_N = GU_TAIL[1] if GU_TAIL else 0
CT_TILES = []
_c0 = 0
while _c0 < CAP:
    CT_TILES.append((_c0, min(P, CAP - _c0)))
    _c0 += P
DG = 512             # down-proj free-dim group
F32 = mybir.dt.float32
F16 = mybir.dt.float16
F8 = mybir.dt.float8e4
WNP = np.float16
E4NP = ml_dtypes.float8_e4m3
AF = mybir.ActivationFunctionType
OP = mybir.AluOpType
AX = mybir.AxisListType
PM_DR = mybir.MatmulPerfMode.DoubleRow

# ---- mixed-precision config ----
GU8 = [c == "1" for c in os.environ.get("BASS_GU8", "00000001")]
DN8 = [c == "1" for c in os.environ.get("BASS_DN8", "1111")]
assert len(GU8) == KH and len(DN8) == KH // 2
SW = 32.0            # fp8 weight scale (w*SW quantized to e4m3)
SH = 8.0             # fp8 hh scale
# DoubleRow LDWEIGHTS requires the stationary k-pair dim step % 16 == 0:
# fp8 token-major tiles are padded from CAP to CAP8
CAP8 = ((CAP + 15) // 16) * 16
K8 = [k for p in range(4) if DN8[p] for k in (2 * p, 2 * p + 1)]
K16 = [k for p in range(4) if not DN8[p] for k in (2 * p, 2 * p + 1)]
KD8N, KD16N = len(K8), len(K16)
M8 = [m for m in range(KH) if GU8[m]]
M16 = [m for m in range(KH) if not GU8[m]]
N8M, N16M = len(M8), len(M16)
M8_IDX = {m: i for i, m in enumerate(M8)}
M16_IDX = {m: i for i, m in enumerate(M16)}
# wsel variant per m: u-epilogue multiplier (SH if down-fp8) / (SW if gu-fp8)
_MSCALE = [
    (SH if DN8[m // 2] else 1.0) / (SW if GU8[m] else 1.0) for m in range(KH)
]
VS = sorted(set(_MSCALE))
NV = len(VS)
VIDX = [VS.index(s) for s in _MSCALE]
# xt chunking in k-tiles; fp8 chunks must have even widths/starts so
# DoubleRow k-pairs never straddle a chunk boundary. Pieces are sized
# ~0.5MB: per-DMA completion latency (~1.5us) bounds queue throughput,
# so fewer/bigger transfers beat fine-grained streaming.
XT16_CH = [2, 2, 4, 4, 4]
XT8_CH = [2, 2, 4, 4, 4]

_gate_nc = None
_moe_nc = None
_wprep_cache = {}
LAST_EXEC_NS = {"gate": None, "moe": None}
_TMPDIR = os.environ.get("BASS_KERNEL_TMPDIR")


def _axon_reset():
    """Recover a wedged NeuronCore via the axon client's reset entry
    point. Best-effort."""
    try:
        import ctypes

        lib = ctypes.CDLL("/opt/axon/libaxon_pjrt.so")
        lib.axon_reset.restype = ctypes.c_int64
        lib.axon_reset()
    except Exception:
        pass


_run_counter = [0]


def _run_spmd(nc, in_maps, trace, tag):
    _run_counter[0] += 1
    tag = f"{tag}_{_run_counter[0]}"
    try:
        return run_bass_kernel_spmd(
            nc, in_maps, core_ids=list(range(N_CORES)), trace=trace,
            tmpdir=(_TMPDIR + "/" + tag) if (trace and _TMPDIR) else None,
        )
    except Exception:
        _axon_reset()
        return run_bass_kernel_spmd(
            nc, in_maps, core_ids=list(range(N_CORES)), trace=trace,
            tmpdir=(_TMPDIR + "/" + tag + "_retry") if (trace and _TMPDIR) else None,
        )


def _emit_warmup(nc, sbuf_pool, psum_pool, psum_tag, n_mm, width, bufs=None,
                 act_funcs=(), mm_width=None):
    """Dummy matmuls with no DMA deps: keep the PE busy from the moment its
    preamble barrier clears so the HAM clock gate flips to 8/8 before the
    first real matmul. Also preloads activation tables (act_funcs) so the
    ~1.3us ACT_TABLE_LOAD overlaps the input DMA window instead of
    stalling the first real activation."""
    warm_src = sbuf_pool.tile([P, width], F16, tag="warm_src")
    nc.gpsimd.memset(warm_src[:], 0.0)
    if act_funcs:
        # separate tile: the act-table preload must not add a dependency
        # onto the warm-up matmuls' source
        warm_act = sbuf_pool.tile([1, 2], F32, tag="warm_act")
        nc.gpsimd.memset(warm_act[:], 0.0)
        for fn in act_funcs:
            nc.scalar.activation(warm_act[:, :1], warm_act[:, 1:], fn)
    mw = mm_width or width
    ps = psum_pool.tile([P, width], F32, tag=psum_tag, name="warm_ps", bufs=bufs)
    for _ in range(n_mm):
        nc.tensor.matmul(
            ps[:, :mw], lhsT=warm_src[:, :P], rhs=warm_src[:, :mw],
            start=True, stop=True,
        )


def _build_gate_nc():
    """Gate kernel: per-core 512-token slice -> renormalized top-2 weights.

    Inputs (feature-major, host-transposed f16 layout):
      xst  [P, KD, TPC]  slice of x^T   (xst[p, k, t] = x[t, k*P+p])
      wgt  [P, KD, E]    W_gate^T      (wgt[p, k, e] = W_gate[e, k*P+p])
    Output:
      wout [TPC, E]  w[t, e] = renormalized top-2 weight, 0 if not selected

    scores^T = wgt.T @ x^T with the 8-column gate weight stationary, then
    PE-transposed back to token-major. Top-2 renormalization uses
    w1 = sigmoid(s1 - s2), w2 = sigmoid(s2 - s1) (the softmax denominator
    cancels), batched across all 4 token tiles in an 11-op chain.
    """
    nc = bacc.Bacc(None, target_bir_lowering=False, enable_partition_id=False)
    xst = nc.dram_tensor("xst", [P, KD, TPC], F16, kind="ExternalInput")
    wgt = nc.dram_tensor("wgt", [P, KD, E], F16, kind="ExternalInput")
    wout = nc.dram_tensor("wout", [TPC, E], F32, kind="ExternalOutput")
    NTT = TPC // P  # 4 token tiles

    with tile.TileContext(nc) as tc:
        with (
            tc.tile_pool(name="xp", bufs=1) as xp,
            tc.tile_pool(name="wp", bufs=1) as wp,
            tc.tile_pool(name="psum", bufs=2, space="PSUM") as psum_pool,
            tc.tile_pool(name="v", bufs=2) as vp,
        ):
            _emit_warmup(nc, wp, psum_pool, "warm", 5, 256,
                         act_funcs=(AF.Sigmoid,))
            ident = wp.tile([P, P], F32)
            make_identity(nc, ident[:])
            wgt_sb = wp.tile([P, KD, E], F16)
            nc.sync.dma_start(wgt_sb[:], wgt[:])
            HP = P // 2
            # xst chunks split across both HWDGE queues by partition
            # halves (disjoint SDMA engine sets); small leading chunks
            # start the matmul stream early, bigger trailing ones
            # amortize the ~1.5us per-DMA completion latency
            CH = [2, 2, 4, 4, 4]
            xst_ch = []
            k0 = 0
            for ci, w in enumerate(CH):
                t = xp.tile([P, w, TPC], F16, tag=f"xst{ci}", name=f"xst{ci}")
                s = xst[:, k0:k0 + w, :]
                nc.sync.dma_start(t[0:HP], s[0:HP])
                nc.scalar.dma_start(t[HP:P], s[HP:P])
                xst_ch.append((t, k0, w))
                k0 += w

            # scores^T [E, TPC], contraction over D in 16 k-tiles
            ps_st = psum_pool.tile([E, TPC], F32, tag="ps_st")
            for k in range(KD):
                t, k0, w = next(c for c in xst_ch if c[1] <= k < c[1] + c[2])
                nc.tensor.matmul(
                    ps_st[:],
                    lhsT=wgt_sb[:, k, :],
                    rhs=t[:, k - k0, :],
                    start=(k == 0),
                    stop=(k == KD - 1),
                )
            st_sb = vp.tile([E, TPC], F32, tag="st")
            nc.vector.tensor_copy(st_sb[:], ps_st[:])

            # transpose back to token-major: sc [P, NTT, E]
            sc = vp.tile([P, NTT, E], F32, tag="sc")
            for tt in range(NTT):
                ps = psum_pool.tile([P, E], F32, tag="scores")
                nc.tensor.transpose(
                    ps[:], st_sb[:, tt * P:(tt + 1) * P], ident[:E, :E]
                )
                nc.vector.tensor_copy(sc[:, tt, :], ps[:])

            # batched top-2 chain over [P, NTT, E]
            m1 = vp.tile([P, NTT, 1], F32, tag="m1")
            nc.vector.tensor_reduce(m1[:], sc[:], op=OP.max, axis=AX.X)
            mask1 = vp.tile([P, NTT, E], F32, tag="mask1")
            nc.vector.tensor_tensor(
                mask1[:], sc[:], m1[:].broadcast_to([P, NTT, E]), op=OP.is_equal
            )
            # sc2 = sc - 1e30*mask1 (knock out the max) in one op
            sc2 = vp.tile([P, NTT, E], F32, tag="sc2")
            nc.vector.scalar_tensor_tensor(
                sc2[:], mask1[:], -1e30, sc[:], op0=OP.mult, op1=OP.add
            )
            m2 = vp.tile([P, NTT, 1], F32, tag="m2")
            nc.vector.tensor_reduce(m2[:], sc2[:], op=OP.max, axis=AX.X)
            mask2 = vp.tile([P, NTT, E], F32, tag="mask2")
            nc.vector.tensor_tensor(
                mask2[:], sc2[:], m2[:].broadcast_to([P, NTT, E]), op=OP.is_equal
            )
            d12 = vp.tile([P, NTT, 1], F32, tag="d12")
            nc.vector.tensor_tensor(d12[:], m1[:], m2[:], op=OP.subtract)
            w1 = vp.tile([P, NTT, 1], F32, tag="w1")
            nc.scalar.activation(w1[:], d12[:], AF.Sigmoid)
            w2 = vp.tile([P, NTT, 1], F32, tag="w2")
            nc.scalar.activation(w2[:], d12[:], AF.Sigmoid, scale=-1.0)
            o1 = vp.tile([P, NTT, E], F32, tag="o1")
            nc.vector.tensor_tensor(
                o1[:], mask1[:], w1[:].broadcast_to([P, NTT, E]), op=OP.mult
            )
            w_all = vp.tile([P, NTT, E], F32, tag="w_all")
            nc.vector.tensor_tensor(
                w_all[:], mask2[:], w2[:].broadcast_to([P, NTT, E]), op=OP.mult
            )
            nc.vector.tensor_tensor(w_all[:], w_all[:], o1[:], op=OP.add)
            nc.scalar.dma_start(
                wout.rearrange("(tt p) e -> p tt e", p=P), w_all[:]
            )
    nc.compile()
    return nc


def _build_moe_nc():
    """Expert FFN kernel: out[c, :] = (silu(x_c @ Wg) * (x_c @ Wu) * wsel[c]) @ Wd.

    Inputs (host-prepared feature/contraction-major layouts; fp8 weights
    pre-scaled by SW, descaled on device):
      xt16   [P, KD, CAP]      f16  gathered tokens (if any fp16 m-tile)
      xt8    [P, KD, CAP]      f8   e4m3(x) (if any fp8 m-tile)
      wg16   [N16M, P, KD, P]  f16  gate-proj columns for fp16 m-tiles
      wg8    [N8M, P, KD, P]   f8   e4m3(SW*w) for fp8 m-tiles
      wu16/wu8                 likewise for up-proj
      wd16   [P, KD16N, D]     f16  down-proj k-slices for fp16 pairs
      wd8    [P, KD8N, D]      f8   e4m3(SW*wd) for fp8 pairs
      wselv  [P, NV, CAP]      f16  routing weight * per-m epilogue scale
    Output:
      out    [CAP, D] f16
    """
    nc = bacc.Bacc(None, target_bir_lowering=False, enable_partition_id=False)
    xt16 = nc.dram_tensor("xt16", [P, KD, CAP], F16, kind="ExternalInput") \
        if N16M else None
    xt8 = nc.dram_tensor("xt8", [P, KD, CAP8], F8, kind="ExternalInput") \
        if N8M else None
    # gate+up weight columns stacked per m-tile: one ~1MB DMA pair per m
    wgu16 = nc.dram_tensor("wgu16", [N16M, P, 2, KD, P], F16,
                           kind="ExternalInput") if N16M else None
    wgu8 = nc.dram_tensor("wgu8", [N8M, P, 2, KD, P], F8,
                          kind="ExternalInput") if N8M else None
    wd16 = nc.dram_tensor("wd16", [P, KD16N, D], F16, kind="ExternalInput") \
        if KD16N else None
    wd8 = nc.dram_tensor("wd8", [P, KD8N, D], F8, kind="ExternalInput") \
        if KD8N else None
    wselv = nc.dram_tensor("wselv", [P, NV, CAP], F16, kind="ExternalInput")
    out = nc.dram_tensor("out", [CAP, D], F16, kind="ExternalOutput")

    with tile.TileContext(nc) as tc:
        with (
            tc.tile_pool(name="sb", bufs=1) as sb,
            tc.tile_pool(name="ps", bufs=1, space="PSUM") as ps_pool,
        ):
            misc = sb
            psum_pool = pgu_pool = ps_pool
            # warm-up psum shares the ps_tg tag: all 8 PSUM banks go to
            # ps_g/ps_u/ps_tg/ps_tu at bufs=2 (tail groups double-buffered
            # kills the ~0.8us stall at each m-tile boundary)
            _emit_warmup(nc, misc, ps_pool, "ps_tg", 18, 512, bufs=2,
                         act_funcs=(AF.Silu,), mm_width=256)

            HP = P // 2

            def dma2(dst, src):
                nc.sync.dma_start(dst[0:HP], src[0:HP])
                nc.scalar.dma_start(dst[HP:P], src[HP:P])

            def w_cols(m, split=False):
                if GU8[m]:
                    src, dt8, tag = wgu8[M8_IDX[m]], F8, "8"
                else:
                    src, dt8, tag = wgu16[M16_IDX[m]], F16, "16"
                t = sb.tile([P, 2, KD, P], dt8, tag=f"wcol{tag}",
                            name=f"wcol{tag}_{m}", bufs=4)
                if split:
                    # m=0 on cold queues: land the gate half first so the
                    # g-matmuls can start before the up half arrives
                    dma2(t[:, 0], src[:, 0])
                    dma2(t[:, 1], src[:, 1])
                else:
                    dma2(t[:], src)
                return t[:, 0], t[:, 1]

            wg_c0, wu_c0 = w_cols(0, split=True)

            # xt chunk tiles; DMAs are placed individually: early chunks on
            # the two HWDGE queues in consumption order, the big tail
            # chunks and the late-consumed fp8 tokens on the gpsimd SWDGE
            # queue (3rd issue stream; all three share the 16 SDMA engines
            # but issue windows no longer serialize)
            def xt_alloc(dt, tag, width, chunks):
                ch = []
                k0 = 0
                for ci, w in enumerate(chunks):
                    t = sb.tile([P, w, width], dt, tag=f"{tag}{ci}",
                                name=f"{tag}{ci}")
                    ch.append((t, k0, w))
                    k0 += w
                return ch

            def xt_dma(dram, ch, ci, gp=False):
                t, k0, w = ch[ci]
                if gp:
                    nc.gpsimd.dma_start(t[:], dram[:, k0:k0 + w, :])
                else:
                    dma2(t[:], dram[:, k0:k0 + w, :])

            first8 = GU8[0]
            xt8_ch = xt16_ch = None
            wsel_sb = misc.tile([P, NV, CAP], F16, tag="wsel")
            if first8:
                xt8_ch = xt_alloc(F8, "xt8", CAP8, XT8_CH)
                for ci in range(len(XT8_CH)):
                    xt_dma(xt8, xt8_ch, ci)
                dma2(wsel_sb[:], wselv[:])
                if N16M:
                    xt16_ch = xt_alloc(F16, "xt16", CAP, XT16_CH)
                    for ci in range(len(XT16_CH)):
                        xt_dma(xt16, xt16_ch, ci)
            else:
                xt16_ch = xt_alloc(F16, "xt16", CAP, XT16_CH)
                for ci in range(len(XT16_CH)):
                    xt_dma(xt16, xt16_ch, ci)
                if N8M:
                    # fp8 tokens are consumed late (fp8 m-tiles sit at the
                    # end): spread their DMAs across the m-loop on SWDGE
                    xt8_ch = xt_alloc(F8, "xt8", CAP8, XT8_CH)
            xt8_next = [0]

            hh8 = sb.tile([P, KD8N, CAP8], F8, name="hh8") if KD8N else None
            hh16 = sb.tile([P, KD16N, CAP], F16, name="hh16") if KD16N else None
            wd8_sb = sb.tile([P, KD8N, D], F8, tag="wd8_sb", name="wd8_sb") \
                if KD8N else None
            wd16_sb = sb.tile([P, KD16N, D], F16, tag="wd16_sb", name="wd16_sb") \
                if KD16N else None

            def find_ch(ch, k):
                return next((t, k0, w) for (t, k0, w) in ch if k0 <= k < k0 + w)

            # ---- gate/up projections + silu*mul*wsel, feature-major ----
            for m in range(KH):
                if m == 0:
                    wg_col, wu_col = wg_c0, wu_c0
                else:
                    wg_col, wu_col = w_cols(m)
                    if m == 1 and not first8:
                        dma2(wsel_sb[:], wselv[:])
                # stage the down-proj weights and late-consumed fp8 tokens
                # mid-stream, in quarter slices so no single piece
                # head-of-line-blocks the next m-tile's weight columns
                if 3 <= m <= 6 and KD8N:
                    q = m - 3
                    s0 = KD8N * q // 4
                    s1 = KD8N * (q + 1) // 4
                    if s1 > s0:
                        dma2(wd8_sb[:, s0:s1, :], wd8[:, s0:s1, :])
                if 3 <= m <= 6 and KD16N:
                    q = m - 3
                    s0 = KD16N * q // 4
                    s1 = KD16N * (q + 1) // 4
                    if s1 > s0:
                        dma2(wd16_sb[:, s0:s1, :], wd16[:, s0:s1, :])
                if not first8 and N8M:
                    # one chunk per iteration, all in place before the
                    # first fp8 m-tile needs them
                    while xt8_next[0] < len(XT8_CH) and (
                        xt8_next[0] <= m or m == M8[0] - 1
                    ):
                        xt_dma(xt8, xt8_ch, xt8_next[0])
                        xt8_next[0] += 1
                is8 = GU8[m]
                xch = xt8_ch if is8 else xt16_ch
                ps_g = [
                    pgu_pool.tile([P, 512], F32, tag="ps_g", name=f"ps_g{gi}", bufs=2)
                    for gi in range(len(GU_FULL))
                ]
                ps_u = [
                    pgu_pool.tile([P, 512], F32, tag="ps_u", name=f"ps_u{gi}", bufs=2)
                    for gi in range(len(GU_FULL))
                ]
                if GU_TAIL:
                    # full-bank tiles so these land in separate PSUM banks:
                    # interleaved accumulation groups sharing one bank
                    # corrupt each other (per-bank clear)
                    ps_tg = psum_pool.tile([P, 512], F32, tag="ps_tg", bufs=2)
                    ps_tu = psum_pool.tile([P, 512], F32, tag="ps_tu", bufs=2)
                # fp8 m-tiles: DoubleRow over 8 k-pairs for the 512-wide
                # groups; the 40-token tail group runs plain fp8 (DoubleRow
                # disables FWL and loses below FD~128). fp16 m-tiles: 16 ks.
                nsteps = KD // 2 if is8 else KD

                def _mm(which, gi, step):
                    w_col = wg_col if which == "g" else wu_col
                    if gi == "T":
                        dst = (ps_tg if which == "g" else ps_tu)[:, :GU_TAIL_N]
                        c0, cn = GU_TAIL[0], GU_TAIL_N
                        for k in ((2 * step, 2 * step + 1) if is8 else (step,)):
                            t, k0, _ = find_ch(xch, k)
                            nc.tensor.matmul(
                                dst,
                                lhsT=w_col[:, k, :],
                                rhs=t[:, k - k0, c0:c0 + cn],
                                start=(k == 0),
                                stop=(k == KD - 1),
                            )
                        return
                    dst = (ps_g if which == "g" else ps_u)[gi][:, :512]
                    c0, cn = GU_FULL[gi]
                    if is8:
                        k = 2 * step
                        t, k0, _ = find_ch(xch, k)
                        nc.tensor.matmul(
                            dst,
                            lhsT=w_col[:, k:k + 2, :],
                            rhs=t[:, k - k0:k - k0 + 2, c0:c0 + cn],
                            start=(step == 0),
                            stop=(step == nsteps - 1),
                            perf_mode=PM_DR,
                        )
                    else:
                        k = step
                        t, k0, _ = find_ch(xch, k)
                        nc.tensor.matmul(
                            dst,
                            lhsT=w_col[:, k, :],
                            rhs=t[:, k - k0, c0:c0 + cn],
                            start=(step == 0),
                            stop=(step == nsteps - 1),
                        )

                tail = [("g", "T"), ("u", "T")] if GU_TAIL else []
                for step in range(nsteps):
                    if step == 0:
                        # tail psum (single buffer) is still being read by
                        # the previous m's silu chain: issue its MMs last
                        order = [("g", 0), ("g", 1), ("u", 0), ("u", 1)] + tail
                    elif step == nsteps - 1:
                        # finish group 0 (g then u) first so its silu chain
                        # starts while the remaining matmuls stream
                        order = [("g", 0), ("u", 0), ("g", 1), ("u", 1)] + tail
                    else:
                        order = [("g", 0), ("g", 1)] + tail[:1] + \
                                [("u", 0), ("u", 1)] + tail[1:]
                    for which, gi in order:
                        _mm(which, gi, step)
                gu_parts = [
                    (c0, cn, ps_g[gi][:, :cn], ps_u[gi][:, :cn])
                    for gi, (c0, cn) in enumerate(GU_FULL)
                ]
                if GU_TAIL:
                    gu_parts.append((
                        GU_TAIL[0], GU_TAIL_N,
                        ps_tg[:, :GU_TAIL_N], ps_tu[:, :GU_TAIL_N],
                    ))
                dn8 = DN8[m // 2]
                hh_t, hh_s = (hh8, K8.index(m)) if dn8 else (hh16, K16.index(m))
                for c0, cn, pg_ap, pu_ap in gu_parts:
                    tmp = misc.tile([P, 512], F16, tag="silu", bufs=2)
                    nc.scalar.activation(
                        tmp[:, :cn], pg_ap, AF.Silu,
                        scale=(1.0 / SW) if is8 else 1.0,
                    )
                    tmp2 = misc.tile([P, 512], F16, tag="uw", bufs=2)
                    nc.vector.tensor_tensor(
                        tmp2[:, :cn],
                        pu_ap,
                        wsel_sb[:, VIDX[m], c0:c0 + cn],
                        op=OP.mult,
                    )
                    nc.vector.tensor_tensor(
                        hh_t[:, hh_s, c0:c0 + cn],
                        tmp[:, :cn],
                        tmp2[:, :cn],
                        op=OP.mult,
                    )

            # ---- down projection, token-major out [CAP, D] ----
            # ct outer / dgi inner; fp8 and fp16 k-pairs accumulate in
            # separate PSUM banks (different scales), combined in the
            # epilogue. o_row split into left/right half-tiles so the
            # left half DMAs out while the right half computes.
            P8 = list(range(0, KD8N, 2))
            P16 = list(range(0, KD16N, 2))
            for (t0, tn) in CT_TILES:
                o_half = [
                    misc.tile([P, D // 2], F16, tag=f"o{h}", name=f"o{h}", bufs=3)
                    for h in range(2)
                ]
                for dgi in range(D // DG):
                    if KD8N:
                        ps8 = ps_pool.tile([P, DG], F32, tag="ps_g", bufs=2)
                        for j, s in enumerate(P8):
                            nc.tensor.matmul(
                                ps8[:tn, :],
                                lhsT=hh8[:, s:s + 2, t0:t0 + tn],
                                rhs=wd8_sb[:, s:s + 2, dgi * DG:(dgi + 1) * DG],
                                start=(j == 0),
                                stop=(j == len(P8) - 1),
                                perf_mode=PM_DR,
                            )
                    if KD16N:
                        ps16 = ps_pool.tile([P, DG], F32, tag="ps_u", bufs=2)
                        for j, s in enumerate(P16):
                            for kk in (s, s + 1):
                                nc.tensor.matmul(
                                    ps16[:tn, :],
                                    lhsT=hh16[:, kk, t0:t0 + tn],
                                    rhs=wd16_sb[:, kk, dgi * DG:(dgi + 1) * DG],
                                    start=(kk == P16[0]),
                                    stop=(kk == P16[-1] + 1),
                                )
                    dst = o_half[dgi // 2][:tn, (dgi % 2) * DG:(dgi % 2 + 1) * DG]
                    if KD8N and KD16N:
                        nc.vector.scalar_tensor_tensor(
                            dst, ps8[:tn, :], 1.0 / (SH * SW), ps16[:tn, :],
                            op0=OP.mult, op1=OP.add,
                        )
                    elif KD8N:
                        nc.vector.tensor_scalar(
                            dst, ps8[:tn, :], 1.0 / (SH * SW), None, op0=OP.mult
                        )
                    else:
                        nc.vector.tensor_copy(dst, ps16[:tn, :])
                    if dgi % 2 == 1:
                        h = (tn + 1) // 2
                        o = o_half[dgi // 2]
                        dd = out[:, (dgi // 2) * 1024:(dgi // 2 + 1) * 1024]
                        nc.sync.dma_start(dd[t0:t0 + h, :], o[:h, :])
                        nc.scalar.dma_start(dd[t0 + h:t0 + tn, :], o[h:tn, :])
    nc.compile()
    return nc


def _feature_major(a2d, dtype=WNP):
    """[D, N] -> [P, D//P, N] (partition, k-tile, free), contiguous."""
    d, n = a2d.shape
    return np.ascontiguousarray(
        a2d.reshape(d // P, P, n).transpose(1, 0, 2).astype(dtype)
    )


def _host_expert(x_tok, wg_e, wu_e, wd_e):
    """Exact fp32 SwiGLU expert for rare capacity-overflow tokens."""
    g = x_tok @ wg_e
    u = x_tok @ wu_e
    hh = (g / (1.0 + np.exp(-g))) * u
    return hh @ wd_e


def kernel(hidden_states, W_gate, w_gate_proj, w_up_proj, w_down_proj):
    global _gate_nc, _moe_nc
    trace = os.environ.get("BASS_KERNEL_TRACE") == "1"

    hidden_states = np.asarray(hidden_states, dtype=np.float32)
    W_gate = np.asarray(W_gate, dtype=np.float32)
    w_gate_proj = np.asarray(w_gate_proj, dtype=np.float32)
    w_up_proj = np.asarray(w_up_proj, dtype=np.float32)
    w_down_proj = np.asarray(w_down_proj, dtype=np.float32)

    x = np.ascontiguousarray(hidden_states.reshape(T, D))
    xh = x.astype(WNP)
    x8 = x.astype(E4NP) if N8M else None

    if _gate_nc is None:
        _gate_nc = _build_gate_nc()
    if _moe_nc is None:
        _moe_nc = _build_moe_nc()

    # ---- phase 1: gate on device (data-parallel over tokens) ----
    wgt_host = _feature_major(W_gate.T.astype(WNP))  # [P, KD, E]
    in_maps1 = []
    for c in range(N_CORES):
        xs = xh[c * TPC:(c + 1) * TPC]            # [TPC, D]
        in_maps1.append({
            "xst": _feature_major(np.ascontiguousarray(xs.T)),  # [P, KD, TPC]
            "wgt": wgt_host,
        })
    res1 = _run_spmd(_gate_nc, in_maps1, trace, "gate")
    LAST_EXEC_NS["gate"] = res1.exec_time_ns
    w = np.concatenate([r["wout"] for r in res1.results], axis=0)  # [T, E]

    # ---- host dispatch: route tokens to expert cores ----
    in_maps2 = []
    idx_list = []
    overflow = []  # (expert, token idx array) handled exactly on host
    cfg_key = "".join("1" if b else "0" for b in GU8 + DN8)
    for e in range(E):
        idx = np.flatnonzero(w[:, e] > 0.0)
        if len(idx) > CAP:
            overflow.append((e, idx[CAP:]))
            idx = idx[:CAP]
        idx_list.append(idx)
        ne = len(idx)
        im = {}
        if N16M:
            xt16_h = np.zeros((P, KD, CAP), WNP)
            xt16_h[:, :, :ne] = _feature_major(np.ascontiguousarray(xh[idx].T))
            im["xt16"] = xt16_h
        if N8M:
            xt8_h = np.zeros((P, KD, CAP8), E4NP)
            xt8_h[:, :, :ne] = _feature_major(
                np.ascontiguousarray(x8[idx].T), dtype=E4NP
            )
            im["xt8"] = xt8_h
        wsv = np.zeros((NV, CAP), WNP)
        for vi, s in enumerate(VS):
            wsv[vi, :ne] = (w[idx, e] * s).astype(WNP)
        im["wselv"] = np.ascontiguousarray(
            np.broadcast_to(wsv, (P, NV, CAP))
        )
        ck = (
            e, cfg_key, w_gate_proj.ctypes.data, float(w_gate_proj[e, 0, 0]),
            float(w_up_proj[e, 1, 1]), float(w_down_proj[e, 2, 2]),
        )
        if ck not in _wprep_cache:
            cw = {}
            wgf = w_gate_proj[e].reshape(KD, P, KH, P).transpose(2, 1, 0, 3)
            wuf = w_up_proj[e].reshape(KD, P, KH, P).transpose(2, 1, 0, 3)
            wdf = w_down_proj[e].reshape(KH, P, D).transpose(1, 0, 2)
            if N16M:
                # [N, 2, P, KD, P] -> [N, P, 2, KD, P]
                cw["wgu16"] = np.ascontiguousarray(
                    np.stack([wgf[M16], wuf[M16]], axis=1).transpose(0, 2, 1, 3, 4)
                ).astype(WNP)
            if N8M:
                cw["wgu8"] = np.ascontiguousarray(
                    np.stack([wgf[M8] * SW, wuf[M8] * SW], axis=1)
                    .transpose(0, 2, 1, 3, 4)
                ).astype(E4NP)
            if KD16N:
                cw["wd16"] = np.ascontiguousarray(wdf[:, K16, :]).astype(WNP)
            if KD8N:
                cw["wd8"] = np.ascontiguousarray(
                    wdf[:, K8, :] * SW
                ).astype(E4NP)
            _wprep_cache[ck] = cw
        im.update(_wprep_cache[ck])
        in_maps2.append(im)

    # ---- phase 2: expert FFN on device (expert-parallel) ----
    res2 = _run_spmd(_moe_nc, in_maps2, trace, "moe")
    LAST_EXEC_NS["moe"] = res2.exec_time_ns

    # ---- host combine: scatter-add + residual ----
    y = x.copy()
    for e in range(E):
        idx = idx_list[e]
        y[idx] += res2.results[e]["out"][:len(idx)].astype(np.float32)
    for e, idx in overflow:
        y[idx] += w[idx, e:e + 1] * _host_expert(
            x[idx], w_gate_proj[e], w_up_proj[e], w_down_proj[e]
        ).astype(np.float32)
    return y.reshape(B, S, D)
